# revision 1
# baseline (speedup 1.0000x reference)
"""Local (sliding-window causal) attention on 8 Trainium2 NeuronCores.

Problem: B=1, S=4096, D=1024, H=16 heads (hd=64), WINDOW=256.
Sharding: tensor-parallel over heads -- 2 heads per core. Each core computes
q/k/v projections for its 2 heads, windowed softmax attention, and its
partial contribution o_c @ Wo_c. The host sums the 8 partials and adds the
bias terms.

Math notes:
 - score uses (q + bq) . (k + bk); the q.bk and bq.bk terms are constant per
   query row so they drop under softmax -> bk is dropped, bq folded into q.
 - v bias: o = p @ (v + bv) = p @ v + bv (softmax rows sum to 1), so the bv
   contribution to the output is the constant row bv @ Wo, added on host.
 - All matmuls run in float32r (tf32-like, full PE speed at N>=256),
   softmax statistics in f32.

Layouts on device (per core):
 - xT     [1024, 4096]   x transposed (host-provided), streamed in 512-col chunks
 - qT,kT  [128, S(+pad)]  head-dim on partitions (2 heads x 64), seq on free
 - v      [128, 34*128]  34 key blocks of [128 keys, 128 hd2]; first 2 blocks zero
 - oT     [128, 4096]    attention output transposed
 - y      [4096, 1024]   partial output (= oT.T @ Wo_c)

Sliding window: queries processed in super-blocks of 256 with a 512-key
padded window [sb*256-256, sb*256+256). Each 128-query half sees 384
contiguous keys of that window; bands masked additively before exp.
"""

import numpy as np

import concourse.bass as bass
import concourse.tile as tile
from concourse import bacc, mybir
from concourse.bass_utils import run_bass_kernel_spmd

# Problem constants (hardcoded per contract -- kernel.py must be self-contained)
S = 4096
D = 1024
H = 16
HD = 64
WINDOW = 256
N_CORES = 8
HPC = H // N_CORES          # heads per core = 2
DH = HPC * HD               # per-core head dims = 128
PAD = 256                   # zero left-padding of keys
SP = S + PAD                # padded key length = 4352
NEG = -1e9

F32 = mybir.dt.float32
F32R = mybir.dt.float32r

N_SB = S // 256             # 16 query super-blocks
N_QB = S // 128             # 32 query blocks
N_T = S // 512              # 8 projection seq chunks
KC = D // 128               # 8 contraction chunks


def _make_masks():
    """Additive masks [128, 384] for one 128-query half of a super-block.

    Query row qi (0..127) may see window-local columns jcol with
    qi+1 <= jcol <= qi+256 (same for both halves). For the first
    super-block, keys left of the sequence start are also masked:
    half 0 requires jcol >= 256, half 1 requires jcol >= 128.
    """
    qi = np.arange(128)[:, None]
    j = np.arange(384)[None, :]
    base = (j >= qi + 1) & (j <= qi + 256)
    m_g = np.where(base, 0.0, NEG).astype(np.float32)
    m0_h0 = np.where(base & (j >= 256), 0.0, NEG).astype(np.float32)
    m0_h1 = np.where(base & (j >= 128), 0.0, NEG).astype(np.float32)
    return m_g, m0_h0, m0_h1


def build_kernel():
    nc = bacc.Bacc()

    xT = nc.dram_tensor("xT", [D, S], F32, kind="ExternalInput")
    wq = nc.dram_tensor("wq", [D, DH], F32, kind="ExternalInput")
    wk = nc.dram_tensor("wk", [D, DH], F32, kind="ExternalInput")
    wv = nc.dram_tensor("wv", [D, DH], F32, kind="ExternalInput")
    bq = nc.dram_tensor("bq", [DH], F32, kind="ExternalInput")
    wo = nc.dram_tensor("wo", [DH, D], F32, kind="ExternalInput")
    y = nc.dram_tensor("y", [S, D], F32, kind="ExternalOutput")

    m_g, m0_h0, m0_h1 = _make_masks()
    mask_g_d = nc.inline_tensor(m_g, name="mask_g")
    mask0_d = [nc.inline_tensor(m0_h0, name="mask0_h0"),
               nc.inline_tensor(m0_h1, name="mask0_h1")]
    ident_d = nc.inline_tensor(np.eye(128, dtype=np.float32), name="ident")

    scale = 1.0 / float(np.sqrt(HD))

    with tile.TileContext(nc) as tc:
        with (
            tc.tile_pool(name="consts", bufs=1) as consts,
            tc.tile_pool(name="persist", bufs=1) as persist,
            tc.tile_pool(name="xstream", bufs=2) as xstream,
            tc.tile_pool(name="work", bufs=3) as work,
            tc.tile_pool(name="ppool", bufs=2) as ppool,
            tc.tile_pool(name="proj_ps", bufs=2, space="PSUM") as proj_ps,
            tc.tile_pool(name="attn_ps", bufs=3, space="PSUM") as attn_ps,
            tc.tile_pool(name="ot_ps", bufs=1, space="PSUM") as ot_ps,
            tc.tile_pool(name="y_ps", bufs=1, space="PSUM") as y_ps,
        ):
            # ---- constants to SBUF ----
            wq_t = consts.tile([128, KC * DH], F32R, name="wq_t")
            wk_t = consts.tile([128, KC * DH], F32R, name="wk_t")
            wv_t = consts.tile([128, KC * DH], F32R, name="wv_t")
            for (t, d) in ((wq_t, wq), (wk_t, wk), (wv_t, wv)):
                d3 = d.ap().rearrange("(c p) m -> p c m", p=128)
                for c in range(KC):
                    nc.sync.dma_start(t[:, c * DH:(c + 1) * DH], d3[:, c].bitcast(F32R))
            wo_t = consts.tile([DH, D], F32R, name="wo_t")
            nc.sync.dma_start(wo_t, wo.ap().bitcast(F32R))

            mask_g = consts.tile([128, 384], F32, name="mask_g")
            nc.sync.dma_start(mask_g, mask_g_d.ap())
            mask0 = []
            for u in range(2):
                mt = consts.tile([128, 384], F32, name=f"mask0_{u}", tag=f"mask0_{u}")
                nc.sync.dma_start(mt, mask0_d[u].ap())
                mask0.append(mt)
            ident = consts.tile([128, 128], F32R, name="ident")
            nc.sync.dma_start(ident, ident_d.ap().bitcast(F32R))

            bq_t = consts.tile([DH, 1], F32, name="bq_t")
            nc.sync.dma_start(bq_t, bq.ap().rearrange("(p o) -> p o", o=1))
            bqs = consts.tile([DH, 1], F32, name="bqs")
            nc.vector.tensor_scalar_mul(bqs, bq_t, scale)

            # ---- persistent activations ----
            qT = persist.tile([128, S], F32R, name="qT")
            kT = persist.tile([128, SP], F32R, name="kT")
            vv = persist.tile([128, (SP // 128) * 128], F32R, name="vv")
            oT = persist.tile([128, S], F32R, name="oT")
            nc.vector.memset(kT[:, 0:PAD].bitcast(F32), 0.0)
            nc.vector.memset(vv[:, 0:PAD].bitcast(F32), 0.0)

            # ---- projections ----
            for t in range(N_T):
                sl = slice(t * 512, (t + 1) * 512)
                xt = xstream.tile([128, KC, 512], F32R, name="xt")
                for c in range(KC):
                    nc.sync.dma_start(
                        xt[:, c], xT.ap()[c * 128:(c + 1) * 128, sl].bitcast(F32R))

                qps = proj_ps.tile([128, 512], F32, name="pps", tag="pps")
                for c in range(KC):
                    nc.tensor.matmul(qps, wq_t[:, c * DH:(c + 1) * DH], xt[:, c],
                                     start=(c == 0), stop=(c == KC - 1))
                nc.scalar.activation(qT[:, sl], qps,
                                     mybir.ActivationFunctionType.Identity,
                                     bias=bqs, scale=scale)

                kps = proj_ps.tile([128, 512], F32, name="kps", tag="pps")
                for c in range(KC):
                    nc.tensor.matmul(kps, wk_t[:, c * DH:(c + 1) * DH], xt[:, c],
                                     start=(c == 0), stop=(c == KC - 1))
                nc.scalar.copy(kT[:, PAD + t * 512:PAD + (t + 1) * 512], kps)

                vps = proj_ps.tile([128, 512], F32, name="vps", tag="pps")
                for c in range(KC):
                    nc.tensor.matmul(vps, wv_t[:, c * DH:(c + 1) * DH], xt[:, c],
                                     start=(c == 0), stop=(c == KC - 1))
                vt = work.tile([128, 512], F32R, name="vt", tag="vt")
                nc.scalar.copy(vt, vps)
                # transpose [hd2, seq] -> [seq, hd2] blocks into vv
                for b in range(4):
                    tb = proj_ps.tile([128, 128], F32R, name="tb", tag="pps")
                    nc.tensor.transpose(tb, vt[:, b * 128:(b + 1) * 128], ident)
                    blk = 2 + t * 4 + b
                    nc.scalar.copy(vv[:, blk * 128:(blk + 1) * 128], tb)

            # ---- attention ----
            for sb in range(N_SB):
                pT = []
                for h in range(2):
                    pt = ppool.tile([128, 4, 2, 128], F32R, name=f"pT{h}",
                                    tag=f"pT{h}")
                    nc.gpsimd.memset(pt[:, 3, 0].bitcast(F32), 0.0)
                    nc.gpsimd.memset(pt[:, 0, 1].bitcast(F32), 0.0)
                    pT.append(pt)
                for u in range(2):
                    qb = sb * 2 + u
                    qsl = slice(qb * 128, (qb + 1) * 128)
                    wsl = slice(sb * 256 + u * 128, sb * 256 + u * 128 + 384)
                    mask = mask0[u] if sb == 0 else mask_g
                    for h in range(2):
                        hsl = slice(h * 64, (h + 1) * 64)
                        sps = attn_ps.tile([128, 384], F32, name="sps", tag="aps")
                        nc.tensor.matmul(sps, qT[hsl, qsl], kT[hsl, wsl],
                                         start=True, stop=True)
                        sm = work.tile([128, 384], F32, name="sm", tag="sm")
                        nc.vector.scalar_tensor_tensor(
                            sm, sps, 1.0, mask,
                            op0=mybir.AluOpType.mult, op1=mybir.AluOpType.add)
                        p = work.tile([128, 384], F32R, name="p", tag="p")
                        rs = work.tile([128, 1], F32, name="rs", tag="rs")
                        nc.scalar.activation(p, sm,
                                             mybir.ActivationFunctionType.Exp,
                                             accum_out=rs)
                        rc = work.tile([128, 1], F32, name="rc", tag="rc")
                        nc.vector.reciprocal(rc, rs)
                        pn = work.tile([128, 384], F32R, name="pn", tag="pn")
                        nc.vector.tensor_scalar_mul(pn, p, rc)
                        tps = attn_ps.tile([128, 384], F32R, name="tps", tag="aps")
                        for kb3 in range(3):
                            nc.tensor.transpose(tps[:, kb3 * 128:(kb3 + 1) * 128],
                                                pn[:, kb3 * 128:(kb3 + 1) * 128],
                                                ident)
                        # window-local key blocks u..u+2, half u
                        nc.vector.tensor_copy(pT[h][:, u:u + 3, u], tps)

                ot = ot_ps.tile([64, 512], F32, name="ot")
                for h in range(2):
                    for kb in range(4):
                        blk = sb * 2 + kb
                        vsl = vv[:, blk * 128 + h * 64: blk * 128 + h * 64 + 64]
                        nc.tensor.matmul(ot[:, h * 256:h * 256 + 256], vsl,
                                         pT[h][:, kb],
                                         start=(kb == 0), stop=(kb == 3))
                ssl = slice(sb * 256, (sb + 1) * 256)
                nc.scalar.copy(oT[0:64, ssl], ot[:, 0:256])
                nc.scalar.copy(oT[64:128, ssl], ot[:, 256:512])

            # ---- output projection (partial: this core's heads only) ----
            for qb in range(N_QB):
                yps = y_ps.tile([128, 1024], F32, name="yps")
                for nch in range(2):
                    nc.tensor.matmul(yps[:, nch * 512:(nch + 1) * 512],
                                     oT[:, qb * 128:(qb + 1) * 128],
                                     wo_t[:, nch * 512:(nch + 1) * 512],
                                     start=True, stop=True)
                ysb = work.tile([128, 1024], F32, name="ysb", tag="ysb")
                nc.vector.tensor_copy(ysb, yps)
                nc.sync.dma_start(y.ap()[qb * 128:(qb + 1) * 128, :], ysb)

    if not nc.is_finalized():
        nc.finalize()
    return nc


_NC_CACHE = None


def kernel(x, Wq, bq, Wk, bk, Wv, bv, Wo, bo, **_kw):
    global _NC_CACHE
    x = np.asarray(x, dtype=np.float32)
    Wq = np.asarray(Wq, dtype=np.float32)
    Wk = np.asarray(Wk, dtype=np.float32)
    Wv = np.asarray(Wv, dtype=np.float32)
    Wo = np.asarray(Wo, dtype=np.float32)
    bq = np.asarray(bq, dtype=np.float32)
    bv = np.asarray(bv, dtype=np.float32)
    bo = np.asarray(bo, dtype=np.float32)

    B = x.shape[0]
    assert x.shape == (B, S, D) and B == 1

    xT = np.ascontiguousarray(x[0].T)

    in_maps = []
    for c in range(N_CORES):
        csl = slice(c * DH, (c + 1) * DH)
        in_maps.append({
            "xT": xT,
            "wq": np.ascontiguousarray(Wq[:, csl]),
            "wk": np.ascontiguousarray(Wk[:, csl]),
            "wv": np.ascontiguousarray(Wv[:, csl]),
            "bq": np.ascontiguousarray(bq[csl]),
            "wo": np.ascontiguousarray(Wo[csl, :]),
        })

    if _NC_CACHE is None:
        _NC_CACHE = build_kernel()
    res = run_bass_kernel_spmd(_NC_CACHE, in_maps, core_ids=list(range(N_CORES)))

    out = np.zeros((S, D), dtype=np.float32)
    for c in range(N_CORES):
        out += res.results[c]["y"]
    # host-side bias terms: bo plus the bv @ Wo constant row (see header)
    out += (bv @ Wo + bo)[None, :]
    return out.reshape(1, S, D)



# revision 8
# speedup vs baseline: 1.7276x; 1.7276x over previous
"""Local (sliding-window causal) attention on 8 Trainium2 NeuronCores.

Problem: B=1, S=4096, D=1024, H=16 heads (hd=64), WINDOW=256.
Sharding: tensor-parallel over heads -- 2 heads per core. Each core computes
q/k/v projections for its 2 heads, windowed softmax attention, and its
partial contribution o_c @ Wo_c. The host sums the 8 partials and adds the
bias terms.

Key design points (v2):
 - All matmuls in fp16 (1 cycle/row on the PE vs ~4.5 for fp32 HIGH mode);
   accumulation stays fp32 in PSUM. Host casts inputs to fp16.
 - Scores are computed TRANSPOSED: sT[key, q] = kT_block^T @ qT, so the
   exp'd probabilities feed the PV matmul directly -- no PE transposes of P.
 - Causal-window masking is multiplicative 0/1 AFTER exp (scores are small,
   |s| < 3, so exp never overflows). Per 128-query block the 384-key padded
   window splits into 3 key blocks: [lower-strict-tri | all-ones | upper-tri],
   one constant fp16 mask tile for all blocks.
 - Softmax row sums come for free: V blocks carry a 65th column of ones, so
   the PV matmul's 65th output row is sum_k exp(s). Normalization happens on
   the attention output: 1/r is broadcast across partitions with a tiny
   rank-2 PE outer product, then one DVE multiply.
 - v projection is computed directly in [seq, hd] layout by using the xT
   chunk as the matmul weights (out partitions = seq), avoiding transposes.
 - score scale 1/sqrt(hd) folded into Wq host-side; bq added on-device via
   per-partition tensor_scalar; bk dropped (softmax shift invariance);
   bv & bo contributions added host-side (softmax rows sum to 1).

Math notes:
 - score uses (q + bq) . (k + bk); the q.bk and bq.bk terms are constant per
   query row so they drop under softmax -> bk is dropped, bq folded into q.
 - v bias: o = p @ (v + bv) = p @ v + bv (softmax rows sum to 1), so the bv
   contribution to the output is the constant row bv @ Wo, added on host.
"""

import numpy as np

import concourse.bass as bass
import concourse.tile as tile
from concourse import bacc, mybir
from concourse.bass_utils import run_bass_kernel_spmd

# Problem constants (hardcoded per contract -- kernel.py must be self-contained)
S = 4096
D = 1024
H = 16
HD = 64
WINDOW = 256
N_CORES = 8
HPC = H // N_CORES          # heads per core = 2
DH = HPC * HD               # per-core head dims = 128

F16 = mybir.dt.float16
F32 = mybir.dt.float32

N_QB = S // 128             # 32 query blocks (and key blocks)
N_T = S // 512              # 8 projection seq chunks
KC = D // 128               # 8 contraction chunks
VB = HD + 1                 # v block stride: 64 v columns + a ones column


def _make_mask():
    """Multiplicative mask [128, 3*128] fp16 in transposed [key, query] layout.

    Query block qb sees key blocks g = qb-2+kb (kb = 0,1,2). For local key
    row jl and query column qi: kb=0 allows jl > qi, kb=1 allows all,
    kb=2 allows jl <= qi.
    """
    jl = np.arange(128)[:, None]
    qi = np.arange(128)[None, :]
    m = np.ones((128, 384), dtype=np.float16)
    m[:, 0:128] = (jl > qi).astype(np.float16)
    m[:, 256:384] = (jl <= qi).astype(np.float16)
    return m


def build_kernel():
    nc = bacc.Bacc()

    xT = nc.dram_tensor("xT", [D, S], F16, kind="ExternalInput")
    wq = nc.dram_tensor("wq", [D, DH], F16, kind="ExternalInput")
    wk = nc.dram_tensor("wk", [D, DH], F16, kind="ExternalInput")
    wv = nc.dram_tensor("wv", [D, DH], F16, kind="ExternalInput")
    bq = nc.dram_tensor("bq", [DH], F32, kind="ExternalInput")
    wo = nc.dram_tensor("wo", [DH, D], F16, kind="ExternalInput")
    y = nc.dram_tensor("y", [S, D], F16, kind="ExternalOutput")

    mask_d = nc.inline_tensor(_make_mask(), name="mask")
    ind = np.zeros((2, 128), dtype=np.float16)
    ind[0, 0:64] = 1.0
    ind[1, 64:128] = 1.0
    ind_d = nc.inline_tensor(ind, name="ind")

    with tile.TileContext(nc) as tc:
        with (
            nc.allow_low_precision(
                reason="fp16 activations by design; rel-err budget 2e-2"),
            tc.tile_pool(name="consts", bufs=1) as consts,
            tc.tile_pool(name="persist", bufs=1) as persist,
            tc.tile_pool(name="xstream", bufs=2) as xstream,
            tc.tile_pool(name="expp", bufs=4) as expp,
            tc.tile_pool(name="work", bufs=2) as work,
        ):
            # ---- constants to SBUF ----
            wq_t = consts.tile([128, KC * DH], F16, name="wq_t")
            wk_t = consts.tile([128, KC * DH], F16, name="wk_t")
            wv_t = consts.tile([128, KC * DH], F16, name="wv_t")
            for (t, d) in ((wq_t, wq), (wk_t, wk), (wv_t, wv)):
                d3 = d.ap().rearrange("(c p) m -> p c m", p=128)
                for c in range(KC):
                    nc.sync.dma_start(t[:, c * DH:(c + 1) * DH], d3[:, c])
            wo_t = consts.tile([DH, D], F16, name="wo_t")
            nc.sync.dma_start(wo_t, wo.ap())

            mask = consts.tile([128, 384], F16, name="mask")
            nc.sync.dma_start(mask, mask_d.ap())
            indt = consts.tile([2, 128], F16, name="indt")
            nc.sync.dma_start(indt, ind_d.ap())
            bqs = consts.tile([DH, 1], F32, name="bqs")
            nc.sync.dma_start(bqs, bq.ap().rearrange("(p o) -> p o", o=1))

            # ---- persistent activations ----
            qT = persist.tile([128, S], F16, name="qT")
            kT = persist.tile([128, S], F16, name="kT")
            vv = persist.tile([128, 2 * N_QB, VB], F16, name="vv")
            # ones columns for the rowsum rows of PV (v copies leave col 64
            # of each 65-wide block untouched)
            nc.gpsimd.memset(vv, 1.0)

            # ---- projections ----
            with (
                tc.tile_pool(name="proj_ps", bufs=2, space="PSUM") as proj_ps,
                tc.tile_pool(name="vp_ps", bufs=2, space="PSUM") as vp_ps,
            ):
                for t in range(N_T):
                    sl = slice(t * 512, (t + 1) * 512)
                    xt = xstream.tile([128, KC, 512], F16, name="xt")
                    for c in range(KC):
                        nc.sync.dma_start(
                            xt[:, c], xT.ap()[c * 128:(c + 1) * 128, sl])

                    qps = proj_ps.tile([128, 512], F32, name="qps", tag="pps")
                    for c in range(KC):
                        nc.tensor.matmul(qps, wq_t[:, c * DH:(c + 1) * DH],
                                         xt[:, c],
                                         start=(c == 0), stop=(c == KC - 1))
                    nc.vector.tensor_scalar_add(qT[:, sl], qps, bqs)

                    kps = proj_ps.tile([128, 512], F32, name="kps", tag="pps")
                    for c in range(KC):
                        nc.tensor.matmul(kps, wk_t[:, c * DH:(c + 1) * DH],
                                         xt[:, c],
                                         start=(c == 0), stop=(c == KC - 1))
                    nc.vector.tensor_copy(kT[:, sl], kps)

                    # v in [seq, hd] layout: xT chunk as weights
                    for blk in range(4):
                        g = t * 4 + blk
                        vps = vp_ps.tile([128, 2, HD], F32, name="vps",
                                         tag="vps")
                        for c in range(KC):
                            nc.tensor.matmul(
                                vps,
                                xt[:, c, blk * 128:(blk + 1) * 128],
                                wv_t[:, c * DH:(c + 1) * DH],
                                start=(c == 0), stop=(c == KC - 1))
                        nc.vector.tensor_copy(
                            vv[:, 2 * g:2 * g + 2, 0:HD], vps)

            # ---- attention + output projection, software-pipelined ----
            with (
                tc.tile_pool(name="sc_ps", bufs=4, space="PSUM") as sc_ps,
                tc.tile_pool(name="aux_ps", bufs=2, space="PSUM") as aux_ps,
                tc.tile_pool(name="y_ps", bufs=2, space="PSUM") as y_ps,
            ):
                def finish(qb, aux):
                    """Normalize attention output of qb and project it."""
                    rr = work.tile([1, 256], F16, name="rr", tag="rr")
                    nc.vector.reciprocal(rr, aux[64:65, 0:256])
                    # broadcast 1/r_h across partitions: rank-1 outer products
                    # into the spare aux columns (h0 -> 384:512, h1 -> 256:384)
                    bcol = (384, 256)
                    for h in range(2):
                        nc.tensor.matmul(
                            aux[0:64, bcol[h]:bcol[h] + 128],
                            indt[0:1, 0:64], rr[0:1, h * 128:(h + 1) * 128],
                            start=True, stop=True)
                    # tensor_tensor reads at most one PSUM operand: stage the
                    # broadcast columns in SBUF
                    bcs = work.tile([64, 256], F16, name="bcs", tag="bcs")
                    nc.vector.tensor_copy(bcs, aux[0:64, 256:512])
                    onorm = work.tile([128, 128], F16, name="onorm",
                                      tag="onorm")
                    for h in range(2):
                        hs = slice(h * 64, (h + 1) * 64)
                        nc.vector.tensor_mul(
                            onorm[hs, :], aux[0:64, h * 128:(h + 1) * 128],
                            bcs[:, bcol[h] - 256:bcol[h] - 128])
                    ysb = work.tile([128, 1024], F16, name="ysb", tag="ysb")
                    for half in range(2):
                        yp = y_ps.tile([128, 512], F32, name="yp", tag="yp")
                        nc.tensor.matmul(yp, onorm,
                                         wo_t[:, half * 512:(half + 1) * 512],
                                         start=True, stop=True)
                        nc.vector.tensor_copy(
                            ysb[:, half * 512:(half + 1) * 512], yp)
                    nc.sync.dma_start(y.ap()[qb * 128:(qb + 1) * 128, :], ysb)

                prev = None
                for qb in range(N_QB):
                    nv = min(qb + 1, 3)      # valid key blocks
                    kbs = range(3 - nv, 3)
                    c0 = (3 - nv) * 128
                    qsl = slice(qb * 128, (qb + 1) * 128)
                    expms = []
                    for h in range(2):
                        hs = slice(h * 64, (h + 1) * 64)
                        sc = sc_ps.tile([128, 384], F32, name="sc", tag="sc")
                        for kb in kbs:
                            g = qb - 2 + kb
                            nc.tensor.matmul(
                                sc[:, kb * 128:(kb + 1) * 128],
                                kT[hs, g * 128:(g + 1) * 128],
                                qT[hs, qsl], start=True, stop=True)
                        expf = expp.tile([128, 384], F16, name="expf",
                                         tag="expf")
                        nc.scalar.activation(
                            expf[:, c0:], sc[:, c0:],
                            mybir.ActivationFunctionType.Exp)
                        expm = expp.tile([128, 384], F16, name="expm",
                                         tag=f"expm{h}")
                        nc.vector.tensor_mul(expm[:, c0:], expf[:, c0:],
                                             mask[:, c0:])
                        expms.append(expm)

                    if prev is not None:
                        finish(*prev)

                    aux = aux_ps.tile([128, 512], F32, name="aux", tag="aux")
                    for h in range(2):
                        for kb in kbs:
                            g = qb - 2 + kb
                            nc.tensor.matmul(
                                aux[0:VB, h * 128:(h + 1) * 128],
                                vv[:, 2 * g + h, :],
                                expms[h][:, kb * 128:(kb + 1) * 128],
                                start=(kb == 3 - nv), stop=(kb == 2))
                    prev = (qb, aux)
                finish(*prev)

    if not nc.is_finalized():
        nc.finalize()
    return nc


def make_in_maps(x, Wq, bq, Wk, Wv, Wo):
    """Per-core input dict list; host does the fp16 casts and head sharding."""
    scale = 1.0 / float(np.sqrt(HD))
    xT = np.ascontiguousarray(np.asarray(x, np.float32)[0].T.astype(np.float16))
    in_maps = []
    for c in range(N_CORES):
        csl = slice(c * DH, (c + 1) * DH)
        in_maps.append({
            "xT": xT,
            "wq": np.ascontiguousarray(
                (np.asarray(Wq, np.float32)[:, csl] * scale).astype(np.float16)),
            "wk": np.ascontiguousarray(
                np.asarray(Wk, np.float32)[:, csl].astype(np.float16)),
            "wv": np.ascontiguousarray(
                np.asarray(Wv, np.float32)[:, csl].astype(np.float16)),
            "bq": np.ascontiguousarray(
                np.asarray(bq, np.float32)[csl] * scale),
            "wo": np.ascontiguousarray(
                np.asarray(Wo, np.float32)[csl, :].astype(np.float16)),
        })
    return in_maps


_NC_CACHE = None


def kernel(x, Wq, bq, Wk, bk, Wv, bv, Wo, bo, **_kw):
    global _NC_CACHE
    x = np.asarray(x, dtype=np.float32)
    B = x.shape[0]
    assert x.shape == (B, S, D) and B == 1

    in_maps = make_in_maps(x, Wq, bq, Wk, Wv, Wo)

    if _NC_CACHE is None:
        _NC_CACHE = build_kernel()
    res = run_bass_kernel_spmd(_NC_CACHE, in_maps, core_ids=list(range(N_CORES)))

    out = np.zeros((S, D), dtype=np.float32)
    for c in range(N_CORES):
        out += res.results[c]["y"].astype(np.float32)
    # host-side bias terms: bo plus the bv @ Wo constant row (see header)
    bv = np.asarray(bv, dtype=np.float32)
    bo = np.asarray(bo, dtype=np.float32)
    Wo = np.asarray(Wo, dtype=np.float32)
    out += (bv @ Wo + bo)[None, :]
    return out.reshape(1, S, D)


# revision 21
# speedup vs baseline: 2.2459x; 1.3000x over previous
"""Local (sliding-window causal) attention on 8 Trainium2 NeuronCores.

Problem: B=1, S=4096, D=1024, H=16 heads (hd=64), WINDOW=256.
Sharding: tensor-parallel over heads -- 2 heads per core. Each core computes
q/k/v projections for its 2 heads, windowed softmax attention, and its
partial contribution o_c @ Wo_c. The host sums the 8 partials and adds the
bias terms.

Key design points (v2):
 - All matmuls in fp16 (1 cycle/row on the PE vs ~4.5 for fp32 HIGH mode);
   accumulation stays fp32 in PSUM. Host casts inputs to fp16.
 - Scores are computed TRANSPOSED: sT[key, q] = kT_block^T @ qT, so the
   exp'd probabilities feed the PV matmul directly -- no PE transposes of P.
 - Causal-window masking is multiplicative 0/1 AFTER exp (scores are small,
   |s| < 3, so exp never overflows). Per 128-query block the 384-key padded
   window splits into 3 key blocks: [lower-strict-tri | all-ones | upper-tri],
   one constant fp16 mask tile for all blocks.
 - Softmax row sums come for free: V blocks carry a 65th column of ones, so
   the PV matmul's 65th output row is sum_k exp(s). Normalization happens on
   the attention output: 1/r is broadcast across partitions with a tiny
   rank-2 PE outer product, then one DVE multiply.
 - v projection is computed directly in [seq, hd] layout by using the xT
   chunk as the matmul weights (out partitions = seq), avoiding transposes.
 - score scale 1/sqrt(hd) folded into Wq host-side; bq added on-device via
   per-partition tensor_scalar; bk dropped (softmax shift invariance);
   bv & bo contributions added host-side (softmax rows sum to 1).

Math notes:
 - score uses (q + bq) . (k + bk); the q.bk and bq.bk terms are constant per
   query row so they drop under softmax -> bk is dropped, bq folded into q.
 - v bias: o = p @ (v + bv) = p @ v + bv (softmax rows sum to 1), so the bv
   contribution to the output is the constant row bv @ Wo, added on host.
"""

import numpy as np

import concourse.bass as bass
import concourse.tile as tile
from concourse import bacc, mybir
from concourse.bass_utils import run_bass_kernel_spmd

# Problem constants (hardcoded per contract -- kernel.py must be self-contained)
S = 4096
D = 1024
H = 16
HD = 64
WINDOW = 256
N_CORES = 8
HPC = H // N_CORES          # heads per core = 2
DH = HPC * HD               # per-core head dims = 128

F16 = mybir.dt.float16
F32 = mybir.dt.float32
F32R = mybir.dt.float32r

N_QB = S // 128             # 32 query blocks (and key blocks)
N_T = S // 512              # 8 projection seq chunks
KC = D // 128               # 8 contraction chunks
VB = HD + 1                 # v block stride: 64 v columns + a ones column


def _make_mask():
    """Multiplicative mask [128, 3*128] fp16 in transposed [key, query] layout.

    Query block qb sees key blocks g = qb-2+kb (kb = 0,1,2). For local key
    row jl and query column qi: kb=0 allows jl > qi, kb=1 allows all,
    kb=2 allows jl <= qi.
    """
    jl = np.arange(128)[:, None]
    qi = np.arange(128)[None, :]
    m = np.ones((128, 384), dtype=np.float16)
    m[:, 0:128] = (jl > qi).astype(np.float16)
    m[:, 256:384] = (jl <= qi).astype(np.float16)
    return m


def build_kernel():
    nc = bacc.Bacc()

    xT = nc.dram_tensor("xT", [D, S], F16, kind="ExternalInput")
    wq = nc.dram_tensor("wq", [D, DH], F16, kind="ExternalInput")
    wk = nc.dram_tensor("wk", [D, DH], F16, kind="ExternalInput")
    wv = nc.dram_tensor("wv", [D, DH], F16, kind="ExternalInput")
    bq = nc.dram_tensor("bq", [DH], F32, kind="ExternalInput")
    wo = nc.dram_tensor("wo", [DH, D], F16, kind="ExternalInput")
    y = nc.dram_tensor("y", [S, D], F16, kind="ExternalOutput")

    mask_d = nc.inline_tensor(_make_mask(), name="mask")
    ident_d = nc.inline_tensor(np.eye(128, dtype=np.float32), name="ident")

    with tile.TileContext(nc) as tc:
        with (
            nc.allow_low_precision(
                reason="fp16 activations by design; rel-err budget 2e-2"),
            tc.tile_pool(name="consts", bufs=1) as consts,
            tc.tile_pool(name="persist", bufs=1) as persist,
            tc.tile_pool(name="xstream", bufs=2) as xstream,
            tc.tile_pool(name="expp", bufs=4) as expp,
            tc.tile_pool(name="work", bufs=2) as work,
        ):
            # ---- constants to SBUF ----
            wq_t = consts.tile([128, KC * DH], F16, name="wq_t")
            wk_t = consts.tile([128, KC * DH], F16, name="wk_t")
            wv_t = consts.tile([128, KC * DH], F16, name="wv_t")
            for (t, d) in ((wq_t, wq), (wk_t, wk), (wv_t, wv)):
                d3 = d.ap().rearrange("(c p) m -> p c m", p=128)
                for c in range(KC):
                    nc.sync.dma_start(t[:, c * DH:(c + 1) * DH], d3[:, c])
            wo_t = consts.tile([DH, D], F16, name="wo_t")
            nc.sync.dma_start(wo_t, wo.ap())

            mask = consts.tile([128, 384], F16, name="mask")
            nc.sync.dma_start(mask, mask_d.ap())
            ident = consts.tile([128, 128], F32R, name="ident")
            nc.sync.dma_start(ident, ident_d.ap().bitcast(F32R))
            bqs = consts.tile([DH, 1], F32, name="bqs")
            nc.sync.dma_start(bqs, bq.ap().rearrange("(p o) -> p o", o=1))

            # ---- persistent activations ----
            qT = persist.tile([128, S], F16, name="qT")
            kT = persist.tile([128, S], F16, name="kT")
            vv = persist.tile([128, 2 * N_QB, VB], F16, name="vv")
            # ones columns for the rowsum rows of PV (v copies leave col 64
            # of each 65-wide block untouched)
            nc.gpsimd.memset(vv, 1.0)

            # ---- projections ----
            with (
                tc.tile_pool(name="proj_ps", bufs=2, space="PSUM") as proj_ps,
                tc.tile_pool(name="vp_ps", bufs=2, space="PSUM") as vp_ps,
            ):
                for t in range(N_T):
                    sl = slice(t * 512, (t + 1) * 512)
                    xt = xstream.tile([128, KC, 512], F16, name="xt")
                    for c in range(KC):
                        nc.sync.dma_start(
                            xt[:, c], xT.ap()[c * 128:(c + 1) * 128, sl])

                    qps = proj_ps.tile([128, 512], F32, name="qps", tag="pps")
                    for c in range(KC):
                        nc.tensor.matmul(qps, wq_t[:, c * DH:(c + 1) * DH],
                                         xt[:, c],
                                         start=(c == 0), stop=(c == KC - 1))
                    nc.vector.tensor_scalar_add(qT[:, sl], qps, bqs)

                    kps = proj_ps.tile([128, 512], F32, name="kps", tag="pps")
                    for c in range(KC):
                        nc.tensor.matmul(kps, wk_t[:, c * DH:(c + 1) * DH],
                                         xt[:, c],
                                         start=(c == 0), stop=(c == KC - 1))
                    nc.vector.tensor_copy(kT[:, sl], kps)

                    # v in [seq, hd] layout: xT chunk as weights
                    for blk in range(4):
                        g = t * 4 + blk
                        vps = vp_ps.tile([128, 2, HD], F32, name="vps",
                                         tag="vps")
                        for c in range(KC):
                            nc.tensor.matmul(
                                vps,
                                xt[:, c, blk * 128:(blk + 1) * 128],
                                wv_t[:, c * DH:(c + 1) * DH],
                                start=(c == 0), stop=(c == KC - 1))
                        nc.vector.tensor_copy(
                            vv[:, 2 * g:2 * g + 2, 0:HD], vps)

            # ---- attention + output projection, software-pipelined ----
            with (
                tc.tile_pool(name="sc_ps", bufs=2, space="PSUM") as sc_ps,
                tc.tile_pool(name="oa_ps", bufs=2, space="PSUM") as oa_ps,
                tc.tile_pool(name="ot_ps", bufs=2, space="PSUM") as ot_ps,
                tc.tile_pool(name="y_ps", bufs=2, space="PSUM") as y_ps,
            ):
                def finish(qb, oa):
                    """Normalize attention output of qb and project it.

                    oa is [128 q, 2 heads, 65] PSUM: cols 0:64 = unnormalized
                    o, col 64 = softmax row sum r. Normalize per-partition,
                    transpose back to [hd, q] on the PE, project.
                    """
                    invr = work.tile([128, 2], F32, name="invr", tag="invr")
                    nc.vector.reciprocal(invr, oa[:, :, 64:65])
                    onq = work.tile([128, 2, HD], F32R, name="onq", tag="onq")
                    for h in range(2):
                        nc.vector.tensor_scalar(
                            onq[:, h], oa[:, h, 0:HD], invr[:, h:h + 1], None,
                            mybir.AluOpType.mult)
                    otp = ot_ps.tile([128, 128], F32R, name="otp", tag="otp")
                    nc.tensor.transpose(otp, onq.rearrange("p a b -> p (a b)"),
                                        ident)
                    onorm = work.tile([128, 128], F16, name="onorm",
                                      tag="onorm")
                    nc.vector.tensor_copy(onorm, otp.bitcast(F32))
                    ysb = work.tile([128, 1024], F16, name="ysb", tag="ysb")
                    for half in range(2):
                        yp = y_ps.tile([128, 512], F32, name="yp", tag="yp")
                        nc.tensor.matmul(yp, onorm,
                                         wo_t[:, half * 512:(half + 1) * 512],
                                         start=True, stop=True)
                        nc.vector.tensor_copy(
                            ysb[:, half * 512:(half + 1) * 512], yp)
                    nc.sync.dma_start(y.ap()[qb * 128:(qb + 1) * 128, :], ysb)

                prev = None
                for qb in range(N_QB):
                    nv = min(qb + 1, 3)      # valid key blocks
                    kbs = range(3 - nv, 3)
                    c0 = (3 - nv) * 128
                    qsl = slice(qb * 128, (qb + 1) * 128)
                    expms = []
                    for h in range(2):
                        hs = slice(h * 64, (h + 1) * 64)
                        sc = sc_ps.tile([128, 384], F32, name="sc", tag="sc")
                        for kb in kbs:
                            g = qb - 2 + kb
                            nc.tensor.matmul(
                                sc[:, kb * 128:(kb + 1) * 128],
                                kT[hs, g * 128:(g + 1) * 128],
                                qT[hs, qsl], start=True, stop=True)
                        expf = expp.tile([128, 384], F16, name="expf",
                                         tag="expf")
                        nc.scalar.activation(
                            expf[:, c0:], sc[:, c0:],
                            mybir.ActivationFunctionType.Exp)
                        expm = expp.tile([128, 384], F16, name="expm",
                                         tag=f"expm{h}")
                        nc.vector.tensor_mul(expm[:, c0:], expf[:, c0:],
                                             mask[:, c0:])
                        expms.append(expm)

                    if prev is not None:
                        finish(*prev)

                    oa = oa_ps.tile([128, 2, VB], F32, name="oa", tag="oa")
                    for h in range(2):
                        for kb in kbs:
                            g = qb - 2 + kb
                            nc.tensor.matmul(
                                oa[:, h],
                                expms[h][:, kb * 128:(kb + 1) * 128],
                                vv[:, 2 * g + h, :],
                                start=(kb == 3 - nv), stop=(kb == 2))
                    prev = (qb, oa)
                finish(*prev)

    if not nc.is_finalized():
        nc.finalize()
    return nc


def make_in_maps(x, Wq, bq, Wk, Wv, Wo):
    """Per-core input dict list; host does the fp16 casts and head sharding."""
    scale = 1.0 / float(np.sqrt(HD))
    xT = np.ascontiguousarray(np.asarray(x, np.float32)[0].T.astype(np.float16))
    in_maps = []
    for c in range(N_CORES):
        csl = slice(c * DH, (c + 1) * DH)
        in_maps.append({
            "xT": xT,
            "wq": np.ascontiguousarray(
                (np.asarray(Wq, np.float32)[:, csl] * scale).astype(np.float16)),
            "wk": np.ascontiguousarray(
                np.asarray(Wk, np.float32)[:, csl].astype(np.float16)),
            "wv": np.ascontiguousarray(
                np.asarray(Wv, np.float32)[:, csl].astype(np.float16)),
            "bq": np.ascontiguousarray(
                np.asarray(bq, np.float32)[csl] * scale),
            "wo": np.ascontiguousarray(
                np.asarray(Wo, np.float32)[csl, :].astype(np.float16)),
        })
    return in_maps


_NC_CACHE = None


def kernel(x, Wq, bq, Wk, bk, Wv, bv, Wo, bo, **_kw):
    global _NC_CACHE
    x = np.asarray(x, dtype=np.float32)
    B = x.shape[0]
    assert x.shape == (B, S, D) and B == 1

    in_maps = make_in_maps(x, Wq, bq, Wk, Wv, Wo)

    if _NC_CACHE is None:
        _NC_CACHE = build_kernel()
    res = run_bass_kernel_spmd(_NC_CACHE, in_maps, core_ids=list(range(N_CORES)))

    out = np.zeros((S, D), dtype=np.float32)
    for c in range(N_CORES):
        out += res.results[c]["y"].astype(np.float32)
    # host-side bias terms: bo plus the bv @ Wo constant row (see header)
    bv = np.asarray(bv, dtype=np.float32)
    bo = np.asarray(bo, dtype=np.float32)
    Wo = np.asarray(Wo, dtype=np.float32)
    out += (bv @ Wo + bo)[None, :]
    return out.reshape(1, S, D)


# revision 25
# speedup vs baseline: 2.4314x; 1.0826x over previous
"""Local (sliding-window causal) attention on 8 Trainium2 NeuronCores.

Problem: B=1, S=4096, D=1024, H=16 heads (hd=64), WINDOW=256.
Sharding: tensor-parallel over heads -- 2 heads per core. Each core computes
q/k/v projections for its 2 heads, windowed softmax attention, and its
partial contribution o_c @ Wo_c. The host sums the 8 partials and adds the
bias terms.

Key design points (v2):
 - All matmuls in fp16 (1 cycle/row on the PE vs ~4.5 for fp32 HIGH mode);
   accumulation stays fp32 in PSUM. Host casts inputs to fp16.
 - Scores are computed TRANSPOSED: sT[key, q] = kT_block^T @ qT, so the
   exp'd probabilities feed the PV matmul directly -- no PE transposes of P.
 - Causal-window masking is multiplicative 0/1 AFTER exp (scores are small,
   |s| < 3, so exp never overflows). Per 128-query block the 384-key padded
   window splits into 3 key blocks: [lower-strict-tri | all-ones | upper-tri],
   one constant fp16 mask tile for all blocks.
 - Softmax row sums come for free: V blocks carry a 65th column of ones, so
   the PV matmul's 65th output row is sum_k exp(s). Normalization happens on
   the attention output: 1/r is broadcast across partitions with a tiny
   rank-2 PE outer product, then one DVE multiply.
 - v projection is computed directly in [seq, hd] layout by using the xT
   chunk as the matmul weights (out partitions = seq), avoiding transposes.
 - score scale 1/sqrt(hd) folded into Wq host-side; bq added on-device via
   per-partition tensor_scalar; bk dropped (softmax shift invariance);
   bv & bo contributions added host-side (softmax rows sum to 1).

Math notes:
 - score uses (q + bq) . (k + bk); the q.bk and bq.bk terms are constant per
   query row so they drop under softmax -> bk is dropped, bq folded into q.
 - v bias: o = p @ (v + bv) = p @ v + bv (softmax rows sum to 1), so the bv
   contribution to the output is the constant row bv @ Wo, added on host.
"""

import numpy as np

import concourse.bass as bass
import concourse.tile as tile
from concourse import bacc, mybir
from concourse.bass_utils import run_bass_kernel_spmd

# Problem constants (hardcoded per contract -- kernel.py must be self-contained)
S = 4096
D = 1024
H = 16
HD = 64
WINDOW = 256
N_CORES = 8
HPC = H // N_CORES          # heads per core = 2
DH = HPC * HD               # per-core head dims = 128

F16 = mybir.dt.float16
F32 = mybir.dt.float32
F32R = mybir.dt.float32r

N_QB = S // 128             # 32 query blocks (and key blocks)
N_T = S // 512              # 8 projection seq chunks
KC = D // 128               # 8 contraction chunks
VB = HD + 1                 # v block stride: 64 v columns + a ones column


def _make_mask():
    """Multiplicative mask [128, 3*128] fp16 in transposed [key, query] layout.

    Query block qb sees key blocks g = qb-2+kb (kb = 0,1,2). For local key
    row jl and query column qi: kb=0 allows jl > qi, kb=1 allows all,
    kb=2 allows jl <= qi.
    """
    jl = np.arange(128)[:, None]
    qi = np.arange(128)[None, :]
    m = np.ones((128, 384), dtype=np.float16)
    m[:, 0:128] = (jl > qi).astype(np.float16)
    m[:, 256:384] = (jl <= qi).astype(np.float16)
    return m


def build_kernel():
    nc = bacc.Bacc()

    xT = nc.dram_tensor("xT", [D, S], F16, kind="ExternalInput")
    wq = nc.dram_tensor("wq", [D, DH], F16, kind="ExternalInput")
    wk = nc.dram_tensor("wk", [D, DH], F16, kind="ExternalInput")
    wv = nc.dram_tensor("wv", [D, DH], F16, kind="ExternalInput")
    bq = nc.dram_tensor("bq", [DH], F32, kind="ExternalInput")
    wo = nc.dram_tensor("wo", [DH, D], F16, kind="ExternalInput")
    y = nc.dram_tensor("y", [S, D], F16, kind="ExternalOutput")

    mask_d = nc.inline_tensor(_make_mask(), name="mask")
    ident_d = nc.inline_tensor(np.eye(128, dtype=np.float32), name="ident")

    with tile.TileContext(nc) as tc:
        with (
            nc.allow_low_precision(
                reason="fp16 activations by design; rel-err budget 2e-2"),
            tc.tile_pool(name="consts", bufs=1) as consts,
            tc.tile_pool(name="persist", bufs=1) as persist,
            tc.tile_pool(name="xstream", bufs=2) as xstream,
            tc.tile_pool(name="expp", bufs=4) as expp,
            tc.tile_pool(name="work", bufs=2) as work,
        ):
            # ---- constants to SBUF ----
            # spread initial DMA descriptor issue across idle engine queues
            # (serial issue on one queue costs ~650ns per descriptor)
            qs = [nc.sync, nc.gpsimd, nc.scalar]
            qi = 0

            def dma(dst, src):
                nonlocal qi
                qs[qi % len(qs)].dma_start(dst, src)
                qi += 1

            wq_t = consts.tile([128, KC * DH], F16, name="wq_t")
            wk_t = consts.tile([128, KC * DH], F16, name="wk_t")
            wv_t = consts.tile([128, KC * DH], F16, name="wv_t")
            for (t, d) in ((wq_t, wq), (wk_t, wk), (wv_t, wv)):
                d3 = d.ap().rearrange("(c p) m -> p c m", p=128)
                for c in range(KC):
                    dma(t[:, c * DH:(c + 1) * DH], d3[:, c])
            wo_t = consts.tile([DH, D], F16, name="wo_t")
            dma(wo_t, wo.ap())

            mask = consts.tile([128, 384], F16, name="mask")
            dma(mask, mask_d.ap())
            ident = consts.tile([128, 128], F32R, name="ident")
            dma(ident, ident_d.ap().bitcast(F32R))
            bqs = consts.tile([DH, 1], F32, name="bqs")
            dma(bqs, bq.ap().rearrange("(p o) -> p o", o=1))

            # ---- persistent activations ----
            qT = persist.tile([128, S], F16, name="qT")
            kT = persist.tile([128, S], F16, name="kT")
            vv = persist.tile([128, 2 * N_QB, VB], F16, name="vv")
            # ones columns for the rowsum rows of PV (v copies leave col 64
            # of each 65-wide block untouched)
            nc.gpsimd.memset(vv, 1.0)

            # ---- projections ----
            with (
                tc.tile_pool(name="proj_ps", bufs=2, space="PSUM") as proj_ps,
                tc.tile_pool(name="vp_ps", bufs=2, space="PSUM") as vp_ps,
            ):
                for t in range(N_T):
                    sl = slice(t * 512, (t + 1) * 512)
                    xt = xstream.tile([128, KC, 512], F16, name="xt")
                    for c in range(KC):
                        (nc.sync if c % 2 == 0 else nc.gpsimd).dma_start(
                            xt[:, c], xT.ap()[c * 128:(c + 1) * 128, sl])

                    qps = proj_ps.tile([128, 512], F32, name="qps", tag="pps")
                    for c in range(KC):
                        nc.tensor.matmul(qps, wq_t[:, c * DH:(c + 1) * DH],
                                         xt[:, c],
                                         start=(c == 0), stop=(c == KC - 1))
                    nc.vector.tensor_scalar_add(qT[:, sl], qps, bqs)

                    kps = proj_ps.tile([128, 512], F32, name="kps", tag="pps")
                    for c in range(KC):
                        nc.tensor.matmul(kps, wk_t[:, c * DH:(c + 1) * DH],
                                         xt[:, c],
                                         start=(c == 0), stop=(c == KC - 1))
                    nc.vector.tensor_copy(kT[:, sl], kps)

                    # v in [seq, hd] layout: xT chunk as weights
                    for blk in range(4):
                        g = t * 4 + blk
                        vps = vp_ps.tile([128, 2, HD], F32, name="vps",
                                         tag="vps")
                        for c in range(KC):
                            nc.tensor.matmul(
                                vps,
                                xt[:, c, blk * 128:(blk + 1) * 128],
                                wv_t[:, c * DH:(c + 1) * DH],
                                start=(c == 0), stop=(c == KC - 1))
                        nc.vector.tensor_copy(
                            vv[:, 2 * g:2 * g + 2, 0:HD], vps)

            # ---- attention + output projection, software-pipelined ----
            with (
                tc.tile_pool(name="sc_ps", bufs=2, space="PSUM") as sc_ps,
                tc.tile_pool(name="oa_ps", bufs=2, space="PSUM") as oa_ps,
                tc.tile_pool(name="ot_ps", bufs=2, space="PSUM") as ot_ps,
                tc.tile_pool(name="y_ps", bufs=2, space="PSUM") as y_ps,
            ):
                def finish(qb, oa):
                    """Normalize attention output of qb and project it.

                    oa is [128 q, 2 heads, 65] PSUM: cols 0:64 = unnormalized
                    o, col 64 = softmax row sum r. Normalize per-partition,
                    transpose back to [hd, q] on the PE, project.
                    """
                    invr = work.tile([128, 2], F32, name="invr", tag="invr")
                    nc.vector.reciprocal(invr, oa[:, :, 64:65])
                    onq = work.tile([128, 2, HD], F32R, name="onq", tag="onq")
                    for h in range(2):
                        nc.vector.tensor_scalar(
                            onq[:, h], oa[:, h, 0:HD], invr[:, h:h + 1], None,
                            mybir.AluOpType.mult)
                    otp = ot_ps.tile([128, 128], F32R, name="otp", tag="otp")
                    nc.tensor.transpose(otp, onq.rearrange("p a b -> p (a b)"),
                                        ident)
                    onorm = work.tile([128, 128], F16, name="onorm",
                                      tag="onorm")
                    nc.vector.tensor_copy(onorm, otp.bitcast(F32))
                    ysb = work.tile([128, 1024], F16, name="ysb", tag="ysb")
                    for half in range(2):
                        yp = y_ps.tile([128, 512], F32, name="yp", tag="yp")
                        nc.tensor.matmul(yp, onorm,
                                         wo_t[:, half * 512:(half + 1) * 512],
                                         start=True, stop=True)
                        nc.vector.tensor_copy(
                            ysb[:, half * 512:(half + 1) * 512], yp)
                    nc.sync.dma_start(y.ap()[qb * 128:(qb + 1) * 128, :], ysb)

                prev = None
                for qb in range(N_QB):
                    nv = min(qb + 1, 3)      # valid key blocks
                    kbs = range(3 - nv, 3)
                    c0 = (3 - nv) * 128
                    qsl = slice(qb * 128, (qb + 1) * 128)
                    expms = []
                    for h in range(2):
                        hs = slice(h * 64, (h + 1) * 64)
                        sc = sc_ps.tile([128, 384], F32, name="sc", tag="sc")
                        for kb in kbs:
                            g = qb - 2 + kb
                            nc.tensor.matmul(
                                sc[:, kb * 128:(kb + 1) * 128],
                                kT[hs, g * 128:(g + 1) * 128],
                                qT[hs, qsl], start=True, stop=True)
                        expm = expp.tile([128, 384], F16, name="expm",
                                         tag=f"expm{h}")
                        nc.scalar.activation(
                            expm[:, c0:], sc[:, c0:],
                            mybir.ActivationFunctionType.Exp)
                        # kb1 is fully visible; apply the triangular masks to
                        # kb0/kb2 in place (0/1 multiplicative, safe post-exp)
                        if nv == 3:
                            nc.gpsimd.tensor_mul(expm[:, 0:128],
                                                 expm[:, 0:128], mask[:, 0:128])
                        nc.gpsimd.tensor_mul(expm[:, 256:384],
                                             expm[:, 256:384], mask[:, 256:384])
                        expms.append(expm)

                    if prev is not None:
                        finish(*prev)

                    oa = oa_ps.tile([128, 2, VB], F32, name="oa", tag="oa")
                    for h in range(2):
                        for kb in kbs:
                            g = qb - 2 + kb
                            nc.tensor.matmul(
                                oa[:, h],
                                expms[h][:, kb * 128:(kb + 1) * 128],
                                vv[:, 2 * g + h, :],
                                start=(kb == 3 - nv), stop=(kb == 2))
                    prev = (qb, oa)
                finish(*prev)

    if not nc.is_finalized():
        nc.finalize()
    return nc


def make_in_maps(x, Wq, bq, Wk, Wv, Wo):
    """Per-core input dict list; host does the fp16 casts and head sharding."""
    scale = 1.0 / float(np.sqrt(HD))
    xT = np.ascontiguousarray(np.asarray(x, np.float32)[0].T.astype(np.float16))
    in_maps = []
    for c in range(N_CORES):
        csl = slice(c * DH, (c + 1) * DH)
        in_maps.append({
            "xT": xT,
            "wq": np.ascontiguousarray(
                (np.asarray(Wq, np.float32)[:, csl] * scale).astype(np.float16)),
            "wk": np.ascontiguousarray(
                np.asarray(Wk, np.float32)[:, csl].astype(np.float16)),
            "wv": np.ascontiguousarray(
                np.asarray(Wv, np.float32)[:, csl].astype(np.float16)),
            "bq": np.ascontiguousarray(
                np.asarray(bq, np.float32)[csl] * scale),
            "wo": np.ascontiguousarray(
                np.asarray(Wo, np.float32)[csl, :].astype(np.float16)),
        })
    return in_maps


_NC_CACHE = None


def kernel(x, Wq, bq, Wk, bk, Wv, bv, Wo, bo, **_kw):
    global _NC_CACHE
    x = np.asarray(x, dtype=np.float32)
    B = x.shape[0]
    assert x.shape == (B, S, D) and B == 1

    in_maps = make_in_maps(x, Wq, bq, Wk, Wv, Wo)

    if _NC_CACHE is None:
        _NC_CACHE = build_kernel()
    res = run_bass_kernel_spmd(_NC_CACHE, in_maps, core_ids=list(range(N_CORES)))

    out = np.zeros((S, D), dtype=np.float32)
    for c in range(N_CORES):
        out += res.results[c]["y"].astype(np.float32)
    # host-side bias terms: bo plus the bv @ Wo constant row (see header)
    bv = np.asarray(bv, dtype=np.float32)
    bo = np.asarray(bo, dtype=np.float32)
    Wo = np.asarray(Wo, dtype=np.float32)
    out += (bv @ Wo + bo)[None, :]
    return out.reshape(1, S, D)


# revision 30
# speedup vs baseline: 2.4621x; 1.0127x over previous
"""Local (sliding-window causal) attention on 8 Trainium2 NeuronCores.

Problem: B=1, S=4096, D=1024, H=16 heads (hd=64), WINDOW=256.
Sharding: tensor-parallel over heads -- 2 heads per core. Each core computes
q/k/v projections for its 2 heads, windowed softmax attention, and its
partial contribution o_c @ Wo_c. The host sums the 8 partials and adds the
bias terms.

Key design points (v2):
 - All matmuls in fp16 (1 cycle/row on the PE vs ~4.5 for fp32 HIGH mode);
   accumulation stays fp32 in PSUM. Host casts inputs to fp16.
 - Scores are computed TRANSPOSED: sT[key, q] = kT_block^T @ qT, so the
   exp'd probabilities feed the PV matmul directly -- no PE transposes of P.
 - Causal-window masking is multiplicative 0/1 AFTER exp (scores are small,
   |s| < 3, so exp never overflows). Per 128-query block the 384-key padded
   window splits into 3 key blocks: [lower-strict-tri | all-ones | upper-tri],
   one constant fp16 mask tile for all blocks.
 - Softmax row sums come for free: V blocks carry a 65th column of ones, so
   the PV matmul's 65th output row is sum_k exp(s). Normalization happens on
   the attention output: 1/r is broadcast across partitions with a tiny
   rank-2 PE outer product, then one DVE multiply.
 - v projection is computed directly in [seq, hd] layout by using the xT
   chunk as the matmul weights (out partitions = seq), avoiding transposes.
 - score scale 1/sqrt(hd) folded into Wq host-side; bq added on-device via
   per-partition tensor_scalar; bk dropped (softmax shift invariance);
   bv & bo contributions added host-side (softmax rows sum to 1).

Math notes:
 - score uses (q + bq) . (k + bk); the q.bk and bq.bk terms are constant per
   query row so they drop under softmax -> bk is dropped, bq folded into q.
 - v bias: o = p @ (v + bv) = p @ v + bv (softmax rows sum to 1), so the bv
   contribution to the output is the constant row bv @ Wo, added on host.
"""

import numpy as np

import concourse.bass as bass
import concourse.tile as tile
from concourse import bacc, mybir
from concourse.bass_utils import run_bass_kernel_spmd

# Problem constants (hardcoded per contract -- kernel.py must be self-contained)
S = 4096
D = 1024
H = 16
HD = 64
WINDOW = 256
N_CORES = 8
HPC = H // N_CORES          # heads per core = 2
DH = HPC * HD               # per-core head dims = 128

F16 = mybir.dt.float16
F32 = mybir.dt.float32
F32R = mybir.dt.float32r

N_QB = S // 128             # 32 query blocks (and key blocks)
N_T = S // 512              # 8 projection seq chunks
KC = D // 128               # 8 contraction chunks
VB = HD + 1                 # v block stride: 64 v columns + a ones column


def _make_mask():
    """Multiplicative masks [128, 256] fp16 in transposed [key, query] layout.

    Query block qb sees key blocks g = qb-2+kb (kb = 0,1,2): kb=0 allows
    local key jl > query qi, kb=1 allows all (no mask), kb=2 allows
    jl <= qi. The expm tiles use column order [kb1 | kb0 | kb2] so the two
    triangular masks land in one contiguous [128, 256] multiply.
    """
    jl = np.arange(128)[:, None]
    qi = np.arange(128)[None, :]
    m = np.ones((128, 256), dtype=np.float16)
    m[:, 0:128] = (jl > qi).astype(np.float16)
    m[:, 128:256] = (jl <= qi).astype(np.float16)
    return m


# expm/sc column offset per key block kb, in [kb1 | kb0 | kb2] order
COL = {0: 128, 1: 0, 2: 256}


def build_kernel():
    nc = bacc.Bacc()

    xT = nc.dram_tensor("xT", [D, S], F16, kind="ExternalInput")
    wq = nc.dram_tensor("wq", [D, DH], F16, kind="ExternalInput")
    wk = nc.dram_tensor("wk", [D, DH], F16, kind="ExternalInput")
    wv = nc.dram_tensor("wv", [D, DH], F16, kind="ExternalInput")
    bq = nc.dram_tensor("bq", [DH], F32, kind="ExternalInput")
    wo = nc.dram_tensor("wo", [DH, D], F16, kind="ExternalInput")
    y = nc.dram_tensor("y", [S, D], F16, kind="ExternalOutput")

    mask_d = nc.inline_tensor(_make_mask(), name="mask")
    ident_d = nc.inline_tensor(np.eye(128, dtype=np.float32), name="ident")

    with tile.TileContext(nc) as tc:
        with (
            nc.allow_low_precision(
                reason="fp16 activations by design; rel-err budget 2e-2"),
            tc.tile_pool(name="consts", bufs=1) as consts,
            tc.tile_pool(name="persist", bufs=1) as persist,
            tc.tile_pool(name="xstream", bufs=2) as xstream,
            tc.tile_pool(name="expp", bufs=4) as expp,
            tc.tile_pool(name="work", bufs=2) as work,
        ):
            # ---- constants to SBUF ----
            # spread initial DMA descriptor issue across idle engine queues
            # (serial issue on one queue costs ~650ns per descriptor)
            qs = [nc.sync, nc.gpsimd, nc.scalar]
            qi = 0

            def dma(dst, src):
                nonlocal qi
                qs[qi % len(qs)].dma_start(dst, src)
                qi += 1

            wq_t = consts.tile([128, KC * DH], F16, name="wq_t")
            wk_t = consts.tile([128, KC * DH], F16, name="wk_t")
            wv_t = consts.tile([128, KC * DH], F16, name="wv_t")
            for (t, d) in ((wq_t, wq), (wk_t, wk), (wv_t, wv)):
                d3 = d.ap().rearrange("(c p) m -> p c m", p=128)
                for c in range(KC):
                    dma(t[:, c * DH:(c + 1) * DH], d3[:, c])
            wo_t = consts.tile([DH, D], F16, name="wo_t")
            dma(wo_t, wo.ap())

            mask = consts.tile([128, 256], F16, name="mask")
            dma(mask, mask_d.ap())
            ident = consts.tile([128, 128], F32R, name="ident")
            dma(ident, ident_d.ap().bitcast(F32R))
            bqs = consts.tile([DH, 1], F32, name="bqs")
            dma(bqs, bq.ap().rearrange("(p o) -> p o", o=1))

            # ---- persistent activations ----
            qT = persist.tile([128, S], F16, name="qT")
            kT = persist.tile([128, S], F16, name="kT")
            vv = persist.tile([128, 2 * N_QB, VB], F16, name="vv")
            # ones columns for the rowsum rows of PV; v proj fills cols 0:64
            nc.vector.memset(vv[:, :, 64:65], 1.0)

            # ---- projections ----
            with (
                tc.tile_pool(name="proj_ps", bufs=2, space="PSUM") as proj_ps,
                tc.tile_pool(name="vp_ps", bufs=2, space="PSUM") as vp_ps,
            ):
                for t in range(N_T):
                    sl = slice(t * 512, (t + 1) * 512)
                    xt = xstream.tile([128, KC, 512], F16, name="xt")
                    for c in range(KC):
                        (nc.sync if c % 2 == 0 else nc.scalar).dma_start(
                            xt[:, c], xT.ap()[c * 128:(c + 1) * 128, sl])

                    qps = proj_ps.tile([128, 512], F32, name="qps", tag="pps")
                    for c in range(KC):
                        nc.tensor.matmul(qps, wq_t[:, c * DH:(c + 1) * DH],
                                         xt[:, c],
                                         start=(c == 0), stop=(c == KC - 1))
                    nc.vector.tensor_scalar_add(qT[:, sl], qps, bqs)

                    kps = proj_ps.tile([128, 512], F32, name="kps", tag="pps")
                    for c in range(KC):
                        nc.tensor.matmul(kps, wk_t[:, c * DH:(c + 1) * DH],
                                         xt[:, c],
                                         start=(c == 0), stop=(c == KC - 1))
                    nc.vector.tensor_copy(kT[:, sl], kps)

                    # v in [seq, hd] layout: xT chunk as weights
                    for blk in range(4):
                        g = t * 4 + blk
                        vps = vp_ps.tile([128, 2, HD], F32, name="vps",
                                         tag="vps")
                        for c in range(KC):
                            nc.tensor.matmul(
                                vps,
                                xt[:, c, blk * 128:(blk + 1) * 128],
                                wv_t[:, c * DH:(c + 1) * DH],
                                start=(c == 0), stop=(c == KC - 1))
                        nc.vector.tensor_copy(
                            vv[:, 2 * g:2 * g + 2, 0:HD], vps)

            # ---- attention + output projection, software-pipelined ----
            with (
                tc.tile_pool(name="sc_ps", bufs=2, space="PSUM") as sc_ps,
                tc.tile_pool(name="oa_ps", bufs=2, space="PSUM") as oa_ps,
                tc.tile_pool(name="ot_ps", bufs=2, space="PSUM") as ot_ps,
                tc.tile_pool(name="y_ps", bufs=2, space="PSUM") as y_ps,
            ):
                def kbs_of(qb):
                    return range(3 - min(qb + 1, 3), 3)

                def score_stage(qb):
                    """Transposed scores + exp + triangular masks -> expm."""
                    qsl = slice(qb * 128, (qb + 1) * 128)
                    expms = []
                    for h in range(2):
                        hs = slice(h * 64, (h + 1) * 64)
                        sc = sc_ps.tile([128, 384], F32, name="sc", tag="sc")
                        for kb in kbs_of(qb):
                            g = qb - 2 + kb
                            nc.tensor.matmul(
                                sc[:, COL[kb]:COL[kb] + 128],
                                kT[hs, g * 128:(g + 1) * 128],
                                qT[hs, qsl], start=True, stop=True)
                        expm = expp.tile([128, 384], F16, name="expm",
                                         tag=f"expm{h}")
                        # kb1 (cols 0:128) is fully visible; kb0/kb2 get the
                        # 0/1 triangular masks in place (exp never overflows:
                        # |score| < 3). For qb=0 only kb2 exists.
                        if qb == 0:
                            nc.scalar.activation(
                                expm[:, 256:384], sc[:, 256:384],
                                mybir.ActivationFunctionType.Exp)
                            nc.gpsimd.tensor_mul(
                                expm[:, 256:384], expm[:, 256:384],
                                mask[:, 128:256])
                        else:
                            nc.scalar.activation(
                                expm, sc, mybir.ActivationFunctionType.Exp)
                            nc.gpsimd.tensor_mul(
                                expm[:, 128:384], expm[:, 128:384], mask)
                        expms.append(expm)
                    return expms

                def pv_stage(qb, expms):
                    """o_aug[q, h, 0:64] = P@V, col 64 = softmax row sum."""
                    oa = oa_ps.tile([128, 2, VB], F32, name="oa", tag="oa")
                    kbs = kbs_of(qb)
                    for h in range(2):
                        for kb in kbs:
                            g = qb - 2 + kb
                            nc.tensor.matmul(
                                oa[:, h],
                                expms[h][:, COL[kb]:COL[kb] + 128],
                                vv[:, 2 * g + h, :],
                                start=(kb == kbs[0]), stop=(kb == 2))
                    return oa

                def finish(qb, oa):
                    """Normalize attention output of qb and project it.

                    oa is [128 q, 2 heads, 65] PSUM: cols 0:64 = unnormalized
                    o, col 64 = softmax row sum r. Normalize per-partition,
                    transpose back to [hd, q] on the PE, project.
                    """
                    invr = work.tile([128, 2], F32, name="invr", tag="invr")
                    nc.vector.reciprocal(invr, oa[:, :, 64:65])
                    onq = work.tile([128, 2, HD], F32R, name="onq", tag="onq")
                    for h in range(2):
                        nc.vector.tensor_scalar(
                            onq[:, h], oa[:, h, 0:HD], invr[:, h:h + 1], None,
                            mybir.AluOpType.mult)
                    otp = ot_ps.tile([128, 128], F32R, name="otp", tag="otp")
                    nc.tensor.transpose(otp, onq.rearrange("p a b -> p (a b)"),
                                        ident)
                    onorm = work.tile([128, 128], F16, name="onorm",
                                      tag="onorm")
                    nc.vector.tensor_copy(onorm, otp.bitcast(F32))
                    ysb = work.tile([128, 1024], F16, name="ysb", tag="ysb")
                    for half in range(2):
                        yp = y_ps.tile([128, 512], F32, name="yp", tag="yp")
                        nc.tensor.matmul(yp, onorm,
                                         wo_t[:, half * 512:(half + 1) * 512],
                                         start=True, stop=True)
                        nc.vector.tensor_copy(
                            ysb[:, half * 512:(half + 1) * 512], yp)
                        # split the y write: halves travel on separate DMA
                        # queues, halving the drain tail
                        (nc.sync if half == 0 else nc.scalar).dma_start(
                            y.ap()[qb * 128:(qb + 1) * 128,
                                   half * 512:(half + 1) * 512],
                            ysb[:, half * 512:(half + 1) * 512])

                # 3-stage software pipeline: scores(qb) | PV(qb-1) |
                # normalize+project(qb-2) keeps the PE fed while the scalar
                # and vector engines chew the previous blocks
                hist = {}
                for qb in range(N_QB):
                    hist[qb] = [score_stage(qb), None]
                    if qb >= 1:
                        hist[qb - 1][1] = pv_stage(qb - 1, hist[qb - 1][0])
                    if qb >= 2:
                        finish(qb - 2, hist.pop(qb - 2)[1])
                hist[N_QB - 1][1] = pv_stage(N_QB - 1, hist[N_QB - 1][0])
                finish(N_QB - 2, hist[N_QB - 2][1])
                finish(N_QB - 1, hist[N_QB - 1][1])

    if not nc.is_finalized():
        nc.finalize()
    return nc


def make_in_maps(x, Wq, bq, Wk, Wv, Wo):
    """Per-core input dict list; host does the fp16 casts and head sharding."""
    scale = 1.0 / float(np.sqrt(HD))
    xT = np.ascontiguousarray(np.asarray(x, np.float32)[0].T.astype(np.float16))
    in_maps = []
    for c in range(N_CORES):
        csl = slice(c * DH, (c + 1) * DH)
        in_maps.append({
            "xT": xT,
            "wq": np.ascontiguousarray(
                (np.asarray(Wq, np.float32)[:, csl] * scale).astype(np.float16)),
            "wk": np.ascontiguousarray(
                np.asarray(Wk, np.float32)[:, csl].astype(np.float16)),
            "wv": np.ascontiguousarray(
                np.asarray(Wv, np.float32)[:, csl].astype(np.float16)),
            "bq": np.ascontiguousarray(
                np.asarray(bq, np.float32)[csl] * scale),
            "wo": np.ascontiguousarray(
                np.asarray(Wo, np.float32)[csl, :].astype(np.float16)),
        })
    return in_maps


_NC_CACHE = None


def kernel(x, Wq, bq, Wk, bk, Wv, bv, Wo, bo, **_kw):
    global _NC_CACHE
    x = np.asarray(x, dtype=np.float32)
    B = x.shape[0]
    assert x.shape == (B, S, D) and B == 1

    in_maps = make_in_maps(x, Wq, bq, Wk, Wv, Wo)

    if _NC_CACHE is None:
        _NC_CACHE = build_kernel()
    res = run_bass_kernel_spmd(_NC_CACHE, in_maps, core_ids=list(range(N_CORES)))

    out = np.zeros((S, D), dtype=np.float32)
    for c in range(N_CORES):
        out += res.results[c]["y"].astype(np.float32)
    # host-side bias terms: bo plus the bv @ Wo constant row (see header)
    bv = np.asarray(bv, dtype=np.float32)
    bo = np.asarray(bo, dtype=np.float32)
    Wo = np.asarray(Wo, dtype=np.float32)
    out += (bv @ Wo + bo)[None, :]
    return out.reshape(1, S, D)


# revision 34
# speedup vs baseline: 2.7396x; 1.1127x over previous
"""Local (sliding-window causal) attention on 8 Trainium2 NeuronCores.

Problem: B=1, S=4096, D=1024, H=16 heads (hd=64), WINDOW=256.
Sharding: tensor-parallel over heads -- 2 heads per core. Each core computes
q/k/v projections for its 2 heads, windowed softmax attention, and its
partial contribution o_c @ Wo_c. The host sums the 8 partials and adds the
bias terms.

Key design points (v2):
 - All matmuls in fp16 (1 cycle/row on the PE vs ~4.5 for fp32 HIGH mode);
   accumulation stays fp32 in PSUM. Host casts inputs to fp16.
 - Scores are computed TRANSPOSED: sT[key, q] = kT_block^T @ qT, so the
   exp'd probabilities feed the PV matmul directly -- no PE transposes of P.
 - Causal-window masking is multiplicative 0/1 AFTER exp (scores are small,
   |s| < 3, so exp never overflows). Per 128-query block the 384-key padded
   window splits into 3 key blocks: [lower-strict-tri | all-ones | upper-tri],
   one constant fp16 mask tile for all blocks.
 - Softmax row sums come for free: V blocks carry a 65th column of ones, so
   the PV matmul's 65th output row is sum_k exp(s). Normalization happens on
   the attention output: 1/r is broadcast across partitions with a tiny
   rank-2 PE outer product, then one DVE multiply.
 - v projection is computed directly in [seq, hd] layout by using the xT
   chunk as the matmul weights (out partitions = seq), avoiding transposes.
 - score scale 1/sqrt(hd) folded into Wq host-side; bq added on-device via
   per-partition tensor_scalar; bk dropped (softmax shift invariance);
   bv & bo contributions added host-side (softmax rows sum to 1).

Math notes:
 - score uses (q + bq) . (k + bk); the q.bk and bq.bk terms are constant per
   query row so they drop under softmax -> bk is dropped, bq folded into q.
 - v bias: o = p @ (v + bv) = p @ v + bv (softmax rows sum to 1), so the bv
   contribution to the output is the constant row bv @ Wo, added on host.
"""

import numpy as np

import concourse.bass as bass
import concourse.tile as tile
from concourse import bacc, mybir
from concourse.bass_utils import run_bass_kernel_spmd

# Problem constants (hardcoded per contract -- kernel.py must be self-contained)
S = 4096
D = 1024
H = 16
HD = 64
WINDOW = 256
N_CORES = 8
HPC = H // N_CORES          # heads per core = 2
DH = HPC * HD               # per-core head dims = 128

F16 = mybir.dt.float16
F32 = mybir.dt.float32
F32R = mybir.dt.float32r

N_QB = S // 128             # 32 query blocks (and key blocks)
N_T = S // 512              # 8 projection seq chunks
KC = D // 128               # 8 contraction chunks
VB = HD + 1                 # v block stride: 64 v columns + a ones column


def _make_mask():
    """Multiplicative masks [128, 256] fp16 in transposed [key, query] layout.

    Query block qb sees key blocks g = qb-2+kb (kb = 0,1,2): kb=0 allows
    local key jl > query qi, kb=1 allows all (no mask), kb=2 allows
    jl <= qi. The expm tiles use column order [kb1 | kb0 | kb2] so the two
    triangular masks land in one contiguous [128, 256] multiply.
    """
    jl = np.arange(128)[:, None]
    qi = np.arange(128)[None, :]
    m = np.ones((128, 256), dtype=np.float16)
    m[:, 0:128] = (jl > qi).astype(np.float16)
    m[:, 128:256] = (jl <= qi).astype(np.float16)
    return m


# expm/sc column offset per key block kb, in [kb1 | kb0 | kb2] order
COL = {0: 128, 1: 0, 2: 256}


def build_kernel():
    nc = bacc.Bacc()

    xT = nc.dram_tensor("xT", [D, S], F16, kind="ExternalInput")
    wq = nc.dram_tensor("wq", [D, DH], F16, kind="ExternalInput")
    wk = nc.dram_tensor("wk", [D, DH], F16, kind="ExternalInput")
    wv = nc.dram_tensor("wv", [D, DH], F16, kind="ExternalInput")
    bq = nc.dram_tensor("bq", [DH], F32, kind="ExternalInput")
    wo = nc.dram_tensor("wo", [DH, D], F16, kind="ExternalInput")
    y = nc.dram_tensor("y", [S, D], F16, kind="ExternalOutput")

    mask_d = nc.inline_tensor(_make_mask(), name="mask")
    ident_d = nc.inline_tensor(np.eye(128, dtype=np.float32), name="ident")

    with tile.TileContext(nc) as tc:
        with (
            nc.allow_low_precision(
                reason="fp16 activations by design; rel-err budget 2e-2"),
            tc.tile_pool(name="consts", bufs=1) as consts,
            tc.tile_pool(name="persist", bufs=1) as persist,
            tc.tile_pool(name="xstream", bufs=2) as xstream,
            tc.tile_pool(name="expp", bufs=4) as expp,
            tc.tile_pool(name="work", bufs=2) as work,
        ):
            # ---- constants to SBUF ----
            # spread initial DMA descriptor issue across idle engine queues
            # (serial issue on one queue costs ~650ns per descriptor)
            qs = [nc.sync, nc.gpsimd, nc.scalar]
            qi = 0

            def dma(dst, src):
                nonlocal qi
                qs[qi % len(qs)].dma_start(dst, src)
                qi += 1

            wq_t = consts.tile([128, KC * DH], F16, name="wq_t")
            wk_t = consts.tile([128, KC * DH], F16, name="wk_t")
            wv_t = consts.tile([128, KC * DH], F16, name="wv_t")
            for (t, d) in ((wq_t, wq), (wk_t, wk), (wv_t, wv)):
                d3 = d.ap().rearrange("(c p) m -> p c m", p=128)
                for c in range(KC):
                    dma(t[:, c * DH:(c + 1) * DH], d3[:, c])
            wo_t = consts.tile([DH, D], F16, name="wo_t")
            dma(wo_t, wo.ap())

            mask = consts.tile([128, 256], F16, name="mask")
            dma(mask, mask_d.ap())
            ident = consts.tile([128, 128], F32R, name="ident")
            dma(ident, ident_d.ap().bitcast(F32R))
            bqs = consts.tile([DH, 1], F32, name="bqs")
            dma(bqs, bq.ap().rearrange("(p o) -> p o", o=1))

            # ---- persistent activations ----
            qT = persist.tile([128, S], F16, name="qT")
            kT = persist.tile([128, S], F16, name="kT")
            vv = persist.tile([128, 2 * N_QB, VB], F16, name="vv")
            # ones columns for the rowsum rows of PV; v proj fills cols 0:64
            nc.vector.memset(vv[:, :, 64:65], 1.0)

            # ---- projections ----
            with (
                tc.tile_pool(name="proj_ps", bufs=2, space="PSUM") as proj_ps,
                tc.tile_pool(name="vp_ps", bufs=2, space="PSUM") as vp_ps,
            ):
                for t in range(N_T):
                    sl = slice(t * 512, (t + 1) * 512)
                    xt = xstream.tile([128, KC, 512], F16, name="xt")
                    for c in range(KC):
                        (nc.sync if c % 2 == 0 else nc.gpsimd).dma_start(
                            xt[:, c], xT.ap()[c * 128:(c + 1) * 128, sl])

                    qps = proj_ps.tile([128, 512], F32, name="qps", tag="pps")
                    for c in range(KC):
                        nc.tensor.matmul(qps, wq_t[:, c * DH:(c + 1) * DH],
                                         xt[:, c],
                                         start=(c == 0), stop=(c == KC - 1))
                    nc.vector.tensor_scalar_add(qT[:, sl], qps, bqs)

                    kps = proj_ps.tile([128, 512], F32, name="kps", tag="pps")
                    for c in range(KC):
                        nc.tensor.matmul(kps, wk_t[:, c * DH:(c + 1) * DH],
                                         xt[:, c],
                                         start=(c == 0), stop=(c == KC - 1))
                    nc.vector.tensor_copy(kT[:, sl], kps)

                    # v in [seq, hd] layout: xT chunk as weights
                    for blk in range(4):
                        g = t * 4 + blk
                        vps = vp_ps.tile([128, 2, HD], F32, name="vps",
                                         tag="vps")
                        for c in range(KC):
                            nc.tensor.matmul(
                                vps,
                                xt[:, c, blk * 128:(blk + 1) * 128],
                                wv_t[:, c * DH:(c + 1) * DH],
                                start=(c == 0), stop=(c == KC - 1))
                        nc.vector.tensor_copy(
                            vv[:, 2 * g:2 * g + 2, 0:HD], vps)

            # ---- attention + output projection, software-pipelined ----
            with (
                tc.tile_pool(name="sc_ps", bufs=2, space="PSUM") as sc_ps,
                tc.tile_pool(name="oa_ps", bufs=3, space="PSUM") as oa_ps,
                tc.tile_pool(name="y_ps", bufs=2, space="PSUM") as y_ps,
            ):
                def kbs_of(qb):
                    return range(3 - min(qb + 1, 3), 3)

                def score_stage(qb):
                    """Transposed scores + exp + triangular masks -> expm."""
                    qsl = slice(qb * 128, (qb + 1) * 128)
                    expms = []
                    for h in range(2):
                        hs = slice(h * 64, (h + 1) * 64)
                        sc = sc_ps.tile([128, 384], F32, name="sc", tag="sc")
                        for kb in kbs_of(qb):
                            g = qb - 2 + kb
                            nc.tensor.matmul(
                                sc[:, COL[kb]:COL[kb] + 128],
                                kT[hs, g * 128:(g + 1) * 128],
                                qT[hs, qsl], start=True, stop=True)
                        expm = expp.tile([128, 384], F16, name="expm",
                                         tag=f"expm{h}")
                        # kb1 (cols 0:128) is fully visible; kb0/kb2 get the
                        # 0/1 triangular masks in place (exp never overflows:
                        # |score| < 3). For qb=0 only kb2 exists.
                        eng = nc.vector if h == 0 else nc.gpsimd
                        if qb == 0:
                            nc.scalar.activation(
                                expm[:, 256:384], sc[:, 256:384],
                                mybir.ActivationFunctionType.Exp)
                            eng.tensor_mul(
                                expm[:, 256:384], expm[:, 256:384],
                                mask[:, 128:256])
                        else:
                            nc.scalar.activation(
                                expm, sc, mybir.ActivationFunctionType.Exp)
                            eng.tensor_mul(
                                expm[:, 128:384], expm[:, 128:384], mask)
                        expms.append(expm)
                    return expms

                def pv_stage(qb, expms):
                    """o_aug[q, h, 0:64] = P@V, col 64 = softmax row sum.

                    The oa tile is a full PSUM bank: cols 0:130 hold the two
                    o_aug blocks, cols 384:512 are scratch for the transpose.
                    """
                    oa = oa_ps.tile([128, 512], F32, name="oa", tag="oa")
                    oav = oa[:, 0:2 * VB].rearrange("p (a b) -> p a b", b=VB)
                    kbs = kbs_of(qb)
                    for h in range(2):
                        for kb in kbs:
                            g = qb - 2 + kb
                            nc.tensor.matmul(
                                oav[:, h],
                                expms[h][:, COL[kb]:COL[kb] + 128],
                                vv[:, 2 * g + h, :],
                                start=(kb == kbs[0]), stop=(kb == 2))
                    return oa

                def finish(qb, oa):
                    """Normalize attention output of qb and project it."""
                    oav = oa[:, 0:2 * VB].rearrange("p (a b) -> p a b", b=VB)
                    invr = work.tile([128, 2], F32, name="invr", tag="invr")
                    nc.vector.reciprocal(invr, oav[:, :, 64:65])
                    onq = work.tile([128, 2, HD], F32R, name="onq", tag="onq")
                    for h in range(2):
                        nc.vector.tensor_scalar(
                            onq[:, h], oav[:, h, 0:HD], invr[:, h:h + 1], None,
                            mybir.AluOpType.mult)
                    nc.tensor.transpose(oa[:, 384:512].bitcast(F32R),
                                        onq.rearrange("p a b -> p (a b)"),
                                        ident)
                    onorm = work.tile([128, 128], F16, name="onorm",
                                      tag="onorm")
                    nc.vector.tensor_copy(onorm, oa[:, 384:512])
                    ysb = work.tile([128, 1024], F16, name="ysb", tag="ysb")
                    for half in range(2):
                        yp = y_ps.tile([128, 512], F32, name="yp", tag="yp")
                        nc.tensor.matmul(yp, onorm,
                                         wo_t[:, half * 512:(half + 1) * 512],
                                         start=True, stop=True)
                        ysl = slice(half * 512, (half + 1) * 512)
                        if half == 0:
                            nc.scalar.copy(ysb[:, ysl], yp)
                        else:
                            nc.vector.tensor_copy(ysb[:, ysl], yp)
                        # split the y write so the two halves travel on
                        # separate DMA queues, halving the drain tail
                        nc.sync.dma_start(
                            y.ap()[qb * 128:(qb + 1) * 128, ysl],
                            ysb[:, ysl])

                # 4-deep software pipeline: scores(qb) | PV(qb-1) | idle |
                # normalize+project(qb-3) keeps the PE fed while scalar and
                # vector engines chew the previous blocks
                hist = {}
                for qb in range(N_QB):
                    hist[qb] = [score_stage(qb), None]
                    if qb >= 1:
                        hist[qb - 1][1] = pv_stage(qb - 1, hist[qb - 1][0])
                    if qb >= 3:
                        finish(qb - 3, hist.pop(qb - 3)[1])
                hist[N_QB - 1][1] = pv_stage(N_QB - 1, hist[N_QB - 1][0])
                for qb in (N_QB - 3, N_QB - 2, N_QB - 1):
                    finish(qb, hist[qb][1])

    if not nc.is_finalized():
        nc.finalize()
    return nc


def make_in_maps(x, Wq, bq, Wk, Wv, Wo):
    """Per-core input dict list; host does the fp16 casts and head sharding."""
    scale = 1.0 / float(np.sqrt(HD))
    xT = np.ascontiguousarray(np.asarray(x, np.float32)[0].T.astype(np.float16))
    in_maps = []
    for c in range(N_CORES):
        csl = slice(c * DH, (c + 1) * DH)
        in_maps.append({
            "xT": xT,
            "wq": np.ascontiguousarray(
                (np.asarray(Wq, np.float32)[:, csl] * scale).astype(np.float16)),
            "wk": np.ascontiguousarray(
                np.asarray(Wk, np.float32)[:, csl].astype(np.float16)),
            "wv": np.ascontiguousarray(
                np.asarray(Wv, np.float32)[:, csl].astype(np.float16)),
            "bq": np.ascontiguousarray(
                np.asarray(bq, np.float32)[csl] * scale),
            "wo": np.ascontiguousarray(
                np.asarray(Wo, np.float32)[csl, :].astype(np.float16)),
        })
    return in_maps


_NC_CACHE = None


def kernel(x, Wq, bq, Wk, bk, Wv, bv, Wo, bo, **_kw):
    global _NC_CACHE
    x = np.asarray(x, dtype=np.float32)
    B = x.shape[0]
    assert x.shape == (B, S, D) and B == 1

    in_maps = make_in_maps(x, Wq, bq, Wk, Wv, Wo)

    if _NC_CACHE is None:
        _NC_CACHE = build_kernel()
    res = run_bass_kernel_spmd(_NC_CACHE, in_maps, core_ids=list(range(N_CORES)))

    out = np.zeros((S, D), dtype=np.float32)
    for c in range(N_CORES):
        out += res.results[c]["y"].astype(np.float32)
    # host-side bias terms: bo plus the bv @ Wo constant row (see header)
    bv = np.asarray(bv, dtype=np.float32)
    bo = np.asarray(bo, dtype=np.float32)
    Wo = np.asarray(Wo, dtype=np.float32)
    out += (bv @ Wo + bo)[None, :]
    return out.reshape(1, S, D)


# revision 37
# speedup vs baseline: 2.8759x; 1.0498x over previous
"""Local (sliding-window causal) attention on 8 Trainium2 NeuronCores.

Problem: B=1, S=4096, D=1024, H=16 heads (hd=64), WINDOW=256.
Sharding: tensor-parallel over heads -- 2 heads per core. Each core computes
q/k/v projections for its 2 heads, windowed softmax attention, and its
partial contribution o_c @ Wo_c. The host sums the 8 partials and adds the
bias terms.

Key design points (v2):
 - All matmuls in fp16 (1 cycle/row on the PE vs ~4.5 for fp32 HIGH mode);
   accumulation stays fp32 in PSUM. Host casts inputs to fp16.
 - Scores are computed TRANSPOSED: sT[key, q] = kT_block^T @ qT, so the
   exp'd probabilities feed the PV matmul directly -- no PE transposes of P.
 - Causal-window masking is multiplicative 0/1 AFTER exp (scores are small,
   |s| < 3, so exp never overflows). Per 128-query block the 384-key padded
   window splits into 3 key blocks: [lower-strict-tri | all-ones | upper-tri],
   one constant fp16 mask tile for all blocks.
 - Softmax row sums come for free: V blocks carry a 65th column of ones, so
   the PV matmul's 65th output row is sum_k exp(s). Normalization happens on
   the attention output: 1/r is broadcast across partitions with a tiny
   rank-2 PE outer product, then one DVE multiply.
 - v projection is computed directly in [seq, hd] layout by using the xT
   chunk as the matmul weights (out partitions = seq), avoiding transposes.
 - score scale 1/sqrt(hd) folded into Wq host-side; bq added on-device via
   per-partition tensor_scalar; bk dropped (softmax shift invariance);
   bv & bo contributions added host-side (softmax rows sum to 1).

Math notes:
 - score uses (q + bq) . (k + bk); the q.bk and bq.bk terms are constant per
   query row so they drop under softmax -> bk is dropped, bq folded into q.
 - v bias: o = p @ (v + bv) = p @ v + bv (softmax rows sum to 1), so the bv
   contribution to the output is the constant row bv @ Wo, added on host.
"""

import numpy as np

import concourse.bass as bass
import concourse.tile as tile
from concourse import bacc, mybir
from concourse.bass_utils import run_bass_kernel_spmd

# Problem constants (hardcoded per contract -- kernel.py must be self-contained)
S = 4096
D = 1024
H = 16
HD = 64
WINDOW = 256
N_CORES = 8
HPC = H // N_CORES          # heads per core = 2
DH = HPC * HD               # per-core head dims = 128

F16 = mybir.dt.float16
F32 = mybir.dt.float32
F32R = mybir.dt.float32r

N_QB = S // 128             # 32 query blocks (and key blocks)
N_T = S // 512              # 8 projection seq chunks
KC = D // 128               # 8 contraction chunks
VB = HD + 1                 # v block stride: 64 v columns + a ones column


def _make_mask():
    """Multiplicative masks [128, 256] fp16 in transposed [key, query] layout.

    Query block qb sees key blocks g = qb-2+kb (kb = 0,1,2): kb=0 allows
    local key jl > query qi, kb=1 allows all (no mask), kb=2 allows
    jl <= qi. The expm tiles use column order [kb1 | kb0 | kb2] so the two
    triangular masks land in one contiguous [128, 256] multiply.
    """
    jl = np.arange(128)[:, None]
    qi = np.arange(128)[None, :]
    m = np.ones((128, 256), dtype=np.float16)
    m[:, 0:128] = (jl > qi).astype(np.float16)
    m[:, 128:256] = (jl <= qi).astype(np.float16)
    return m


# expm/sc column offset per key block kb, in [kb1 | kb0 | kb2] order
COL = {0: 128, 1: 0, 2: 256}


def build_kernel():
    nc = bacc.Bacc()

    xT = nc.dram_tensor("xT", [D, S], F16, kind="ExternalInput")
    wq = nc.dram_tensor("wq", [D, DH], F16, kind="ExternalInput")
    wk = nc.dram_tensor("wk", [D, DH], F16, kind="ExternalInput")
    wv = nc.dram_tensor("wv", [D, DH], F16, kind="ExternalInput")
    bq = nc.dram_tensor("bq", [DH], F32, kind="ExternalInput")
    wo = nc.dram_tensor("wo", [DH, D], F16, kind="ExternalInput")
    y = nc.dram_tensor("y", [S, D], F16, kind="ExternalOutput")

    mask_d = nc.inline_tensor(_make_mask(), name="mask")
    ident_d = nc.inline_tensor(np.eye(128, dtype=np.float32), name="ident")

    with tile.TileContext(nc) as tc:
        with (
            nc.allow_low_precision(
                reason="fp16 activations by design; rel-err budget 2e-2"),
            tc.tile_pool(name="consts", bufs=1) as consts,
            tc.tile_pool(name="persist", bufs=1) as persist,
            tc.tile_pool(name="xstream", bufs=2) as xstream,
            tc.tile_pool(name="expp", bufs=4) as expp,
            tc.tile_pool(name="work", bufs=2) as work,
        ):
            # ---- constants to SBUF ----
            # First x chunk goes out on sync immediately -- its ~5.6us
            # transfer is the critical path to the first matmul. Constants
            # spread across the other queues meanwhile.
            xt0 = xstream.tile([128, KC, 512], F16, name="xt")
            for c in range(KC):
                nc.sync.dma_start(xt0[:, c], xT.ap()[c * 128:(c + 1) * 128,
                                                     0:512])

            qs = [nc.gpsimd, nc.scalar]
            qi = 0

            def dma(dst, src):
                nonlocal qi
                qs[qi % len(qs)].dma_start(dst, src)
                qi += 1

            wq_t = consts.tile([128, KC * DH], F16, name="wq_t")
            wk_t = consts.tile([128, KC * DH], F16, name="wk_t")
            wv_t = consts.tile([128, KC * DH], F16, name="wv_t")
            for (t, d) in ((wq_t, wq), (wk_t, wk), (wv_t, wv)):
                d3 = d.ap().rearrange("(c p) m -> p c m", p=128)
                for c in range(KC):
                    dma(t[:, c * DH:(c + 1) * DH], d3[:, c])
            wo_t = consts.tile([DH, D], F16, name="wo_t")
            dma(wo_t, wo.ap())

            mask = consts.tile([128, 256], F16, name="mask")
            dma(mask, mask_d.ap())
            ident = consts.tile([128, 128], F32R, name="ident")
            dma(ident, ident_d.ap().bitcast(F32R))
            bqs = consts.tile([DH, 1], F32, name="bqs")
            dma(bqs, bq.ap().rearrange("(p o) -> p o", o=1))

            # ---- persistent activations ----
            qT = persist.tile([128, S], F16, name="qT")
            kT = persist.tile([128, S], F16, name="kT")
            vv = persist.tile([128, 2 * N_QB, VB], F16, name="vv")
            # ones columns for the rowsum rows of PV; v proj fills cols 0:64
            nc.vector.memset(vv[:, :, 64:65], 1.0)

            # ---- projections ----
            with (
                tc.tile_pool(name="proj_ps", bufs=2, space="PSUM") as proj_ps,
                tc.tile_pool(name="vp_ps", bufs=2, space="PSUM") as vp_ps,
            ):
                for t in range(N_T):
                    sl = slice(t * 512, (t + 1) * 512)
                    if t == 0:
                        xt = xt0
                    else:
                        xt = xstream.tile([128, KC, 512], F16, name="xt")
                        for c in range(KC):
                            nc.sync.dma_start(
                                xt[:, c], xT.ap()[c * 128:(c + 1) * 128, sl])

                    qps = proj_ps.tile([128, 512], F32, name="qps", tag="pps")
                    for c in range(KC):
                        nc.tensor.matmul(qps, wq_t[:, c * DH:(c + 1) * DH],
                                         xt[:, c],
                                         start=(c == 0), stop=(c == KC - 1))
                    nc.vector.tensor_scalar_add(qT[:, sl], qps, bqs)

                    kps = proj_ps.tile([128, 512], F32, name="kps", tag="pps")
                    for c in range(KC):
                        nc.tensor.matmul(kps, wk_t[:, c * DH:(c + 1) * DH],
                                         xt[:, c],
                                         start=(c == 0), stop=(c == KC - 1))
                    nc.vector.tensor_copy(kT[:, sl], kps)

                    # v in [seq, hd] layout: xT chunk as weights
                    for blk in range(4):
                        g = t * 4 + blk
                        vps = vp_ps.tile([128, 2, HD], F32, name="vps",
                                         tag="vps")
                        for c in range(KC):
                            nc.tensor.matmul(
                                vps,
                                xt[:, c, blk * 128:(blk + 1) * 128],
                                wv_t[:, c * DH:(c + 1) * DH],
                                start=(c == 0), stop=(c == KC - 1))
                        nc.vector.tensor_copy(
                            vv[:, 2 * g:2 * g + 2, 0:HD], vps)

            # ---- attention + output projection, software-pipelined ----
            with (
                tc.tile_pool(name="sc_ps", bufs=2, space="PSUM") as sc_ps,
                tc.tile_pool(name="oa_ps", bufs=3, space="PSUM") as oa_ps,
                tc.tile_pool(name="y_ps", bufs=2, space="PSUM") as y_ps,
            ):
                def kbs_of(qb):
                    return range(3 - min(qb + 1, 3), 3)

                def score_stage(qb):
                    """Transposed scores + exp + triangular masks -> expm."""
                    qsl = slice(qb * 128, (qb + 1) * 128)
                    expms = []
                    for h in range(2):
                        hs = slice(h * 64, (h + 1) * 64)
                        sc = sc_ps.tile([128, 384], F32, name="sc", tag="sc")
                        for kb in kbs_of(qb):
                            g = qb - 2 + kb
                            nc.tensor.matmul(
                                sc[:, COL[kb]:COL[kb] + 128],
                                kT[hs, g * 128:(g + 1) * 128],
                                qT[hs, qsl], start=True, stop=True)
                        expm = expp.tile([128, 384], F16, name="expm",
                                         tag=f"expm{h}")
                        # kb1 (cols 0:128) is fully visible; kb0/kb2 get the
                        # 0/1 triangular masks in place (exp never overflows:
                        # |score| < 3). For qb=0 only kb2 exists.
                        eng = nc.vector if h == 0 else nc.gpsimd
                        if qb == 0:
                            nc.scalar.activation(
                                expm[:, 256:384], sc[:, 256:384],
                                mybir.ActivationFunctionType.Exp)
                            eng.tensor_mul(
                                expm[:, 256:384], expm[:, 256:384],
                                mask[:, 128:256])
                        else:
                            nc.scalar.activation(
                                expm, sc, mybir.ActivationFunctionType.Exp)
                            eng.tensor_mul(
                                expm[:, 128:384], expm[:, 128:384], mask)
                        expms.append(expm)
                    return expms

                def pv_stage(qb, expms):
                    """o_aug[q, h, 0:64] = P@V, col 64 = softmax row sum.

                    The oa tile is a full PSUM bank: cols 0:130 hold the two
                    o_aug blocks, cols 384:512 are scratch for the transpose.
                    """
                    oa = oa_ps.tile([128, 512], F32, name="oa", tag="oa")
                    oav = oa[:, 0:2 * VB].rearrange("p (a b) -> p a b", b=VB)
                    kbs = kbs_of(qb)
                    for h in range(2):
                        for kb in kbs:
                            g = qb - 2 + kb
                            nc.tensor.matmul(
                                oav[:, h],
                                expms[h][:, COL[kb]:COL[kb] + 128],
                                vv[:, 2 * g + h, :],
                                start=(kb == kbs[0]), stop=(kb == 2))
                    return oa

                def finish(qb, oa):
                    """Normalize attention output of qb and project it."""
                    oav = oa[:, 0:2 * VB].rearrange("p (a b) -> p a b", b=VB)
                    invr = work.tile([128, 2], F32, name="invr", tag="invr")
                    nc.vector.reciprocal(invr, oav[:, :, 64:65])
                    onq = work.tile([128, 2, HD], F32R, name="onq", tag="onq")
                    for h in range(2):
                        nc.vector.tensor_scalar(
                            onq[:, h], oav[:, h, 0:HD], invr[:, h:h + 1], None,
                            mybir.AluOpType.mult)
                    nc.tensor.transpose(oa[:, 384:512].bitcast(F32R),
                                        onq.rearrange("p a b -> p (a b)"),
                                        ident)
                    onorm = work.tile([128, 128], F16, name="onorm",
                                      tag="onorm")
                    nc.vector.tensor_copy(onorm, oa[:, 384:512])
                    ysb = work.tile([128, 1024], F16, name="ysb", tag="ysb")
                    for half in range(2):
                        yp = y_ps.tile([128, 512], F32, name="yp", tag="yp")
                        nc.tensor.matmul(yp, onorm,
                                         wo_t[:, half * 512:(half + 1) * 512],
                                         start=True, stop=True)
                        ysl = slice(half * 512, (half + 1) * 512)
                        if half == 0:
                            nc.scalar.copy(ysb[:, ysl], yp)
                        else:
                            nc.vector.tensor_copy(ysb[:, ysl], yp)
                        # split the y write so the two halves travel on
                        # separate DMA queues, halving the drain tail
                        (nc.sync if half == 0 else nc.gpsimd).dma_start(
                            y.ap()[qb * 128:(qb + 1) * 128, ysl],
                            ysb[:, ysl])

                # 4-deep software pipeline: scores(qb) | PV(qb-1) | idle |
                # normalize+project(qb-3) keeps the PE fed while scalar and
                # vector engines chew the previous blocks
                hist = {}
                for qb in range(N_QB):
                    hist[qb] = [score_stage(qb), None]
                    if qb >= 1:
                        hist[qb - 1][1] = pv_stage(qb - 1, hist[qb - 1][0])
                    if qb >= 3:
                        finish(qb - 3, hist.pop(qb - 3)[1])
                hist[N_QB - 1][1] = pv_stage(N_QB - 1, hist[N_QB - 1][0])
                for qb in (N_QB - 3, N_QB - 2, N_QB - 1):
                    finish(qb, hist[qb][1])

    if not nc.is_finalized():
        nc.finalize()
    return nc


def make_in_maps(x, Wq, bq, Wk, Wv, Wo):
    """Per-core input dict list; host does the fp16 casts and head sharding."""
    scale = 1.0 / float(np.sqrt(HD))
    xT = np.ascontiguousarray(np.asarray(x, np.float32)[0].T.astype(np.float16))
    in_maps = []
    for c in range(N_CORES):
        csl = slice(c * DH, (c + 1) * DH)
        in_maps.append({
            "xT": xT,
            "wq": np.ascontiguousarray(
                (np.asarray(Wq, np.float32)[:, csl] * scale).astype(np.float16)),
            "wk": np.ascontiguousarray(
                np.asarray(Wk, np.float32)[:, csl].astype(np.float16)),
            "wv": np.ascontiguousarray(
                np.asarray(Wv, np.float32)[:, csl].astype(np.float16)),
            "bq": np.ascontiguousarray(
                np.asarray(bq, np.float32)[csl] * scale),
            "wo": np.ascontiguousarray(
                np.asarray(Wo, np.float32)[csl, :].astype(np.float16)),
        })
    return in_maps


_NC_CACHE = None


def kernel(x, Wq, bq, Wk, bk, Wv, bv, Wo, bo, **_kw):
    global _NC_CACHE
    x = np.asarray(x, dtype=np.float32)
    B = x.shape[0]
    assert x.shape == (B, S, D) and B == 1

    in_maps = make_in_maps(x, Wq, bq, Wk, Wv, Wo)

    if _NC_CACHE is None:
        _NC_CACHE = build_kernel()
    res = run_bass_kernel_spmd(_NC_CACHE, in_maps, core_ids=list(range(N_CORES)))

    out = np.zeros((S, D), dtype=np.float32)
    for c in range(N_CORES):
        out += res.results[c]["y"].astype(np.float32)
    # host-side bias terms: bo plus the bv @ Wo constant row (see header)
    bv = np.asarray(bv, dtype=np.float32)
    bo = np.asarray(bo, dtype=np.float32)
    Wo = np.asarray(Wo, dtype=np.float32)
    out += (bv @ Wo + bo)[None, :]
    return out.reshape(1, S, D)
